# revision 7
# baseline (speedup 1.0000x reference)
"""DiT block (Linformer attention + adaLN + MLP) on 8 TRN2 NeuronCores.

Sharding: data-parallel over batch (B=8 -> one batch element per core).
All matmuls run in float32r (full PE speed at free-dim>=256, ~1.6e-4 rel err).

Layout per core (S=2048 tokens, D=1024 features):
 - adaLN in natural [S_p, D_f] layout (bn_stats over free dim), PE-transpose
   x1 into x1T [D_p, S_f]; qT = wq^T @ x1T computed per token-group while
   the x1T tiles rotate.
 - Linformer K/V: k_projT = wk^T @ (x1^T @ Ew) -- the S->K projection is
   applied to x1 first (P_EF = x1^T @ [Ew|Fw]), so the wk/wv matmuls shrink
   by S/K = 8x and k,v are never materialized.
 - scores stay transposed: scoresT_h [K_p, S_f]; softmax denominators come
   from a fused ones-column appended to v_proj (65-row attn@V output);
   normalization uses a ones-matmul partition-broadcast of 1/denom.
 - MLP streams m1w/m2w from a pre-rounded fp32r DRAM copy in 4 token-groups.

SBUF pool lifetimes are split across the left/right allocator stacks so the
cross-phase handoff chain stays LIFO per side.
"""
import contextlib

import numpy as np

import concourse.bass as bass
import concourse.mybir as mybir
import concourse.tile as tile
from concourse import bacc
from concourse.bass import ds, ts
from concourse.bass_utils import run_bass_kernel_spmd
from concourse.masks import make_identity

f32 = mybir.dt.float32
f32r = mybir.dt.float32r
AF = mybir.ActivationFunctionType
OP = mybir.AluOpType

B, S, D, H, K, MLP, ZD = 8, 2048, 1024, 16, 256, 4096, 1024
DH = D // H      # 64
P = 128
SC = S // P      # 16 token chunks of 128
DC = D // P      # 8 feature chunks of 128
NG = 4           # token groups of 512
GS = 512
MC = MLP // P    # 32
KC = K // P      # 2
EPS = 1e-6

W2D = [("wq", D, D), ("wk", D, D), ("wv", D, D), ("wo", D, D),
       ("Ew", S, K), ("Fw", S, K),
       ("h1w", ZD, D), ("g1w", D, D), ("be1w", D, D),
       ("h2w", ZD, D), ("g2w", D, D), ("be2w", D, D),
       ("m1w", D, MLP), ("m2w", MLP, D)]
W1D = [("bq", D), ("bk", D), ("bv", D), ("bo", D), ("Eb", K), ("Fb", K),
       ("h1b", D), ("g1b", D), ("be1b", D), ("h2b", D), ("g2b", D), ("be2b", D),
       ("m1b", MLP), ("m2b", D)]

_cache = {}


def build():
    if "nc" in _cache:
        return _cache["nc"]
    nc = bacc.Bacc("TRN2", target_bir_lowering=False, debug=False, num_devices=8)
    ap = {}
    ap["x"] = nc.dram_tensor("x", [S, D], f32, kind="ExternalInput").ap()
    ap["z"] = nc.dram_tensor("z", [1, ZD], f32, kind="ExternalInput").ap()
    for nm, a, b in W2D:
        ap[nm] = nc.dram_tensor(nm, [a, b], f32, kind="ExternalInput").ap()
    for nm, a in W1D:
        ap[nm] = nc.dram_tensor(nm, [a], f32, kind="ExternalInput").ap()
    out = nc.dram_tensor("out", [S, D], f32, kind="ExternalOutput").ap()
    with tile.TileContext(nc, trace_sim=False) as tc:
        _emit(nc, tc, ap, out)
    nc.compile()
    _cache["nc"] = nc
    return nc


def _emit(nc, tc, ap, out):
    ctx = contextlib.ExitStack()
    with ctx:
        # ---------- whole-kernel pools ----------
        const = ctx.enter_context(tc.tile_pool(name="const", bufs=1))
        rows = ctx.enter_context(tc.tile_pool(name="rows", bufs=1))
        cols = ctx.enter_context(tc.tile_pool(name="cols", bufs=1))
        dram = ctx.enter_context(tc.tile_pool(name="dram", bufs=1, space="DRAM"))

        attn_sc = dram.tile([S, D], f32, tag="attn_sc", name="attn_sc")
        m1w_r = dram.tile([D, MLP], f32r, tag="m1w_r", name="m1w_r")
        m2w_r = dram.tile([MLP, D], f32r, tag="m2w_r", name="m2w_r")

        ident_f = const.tile([P, P], f32, tag="ident_f", name="ident_f")
        make_identity(nc, ident_f)
        ident_r = const.tile([P, P], f32r, tag="ident_r", name="ident_r")
        nc.vector.tensor_copy(ident_r[:], ident_f[:])
        eps_t = const.tile([P, 1], f32, tag="eps", name="eps")
        nc.vector.memset(eps_t[:], EPS)
        ones_f = const.tile([P, 16], f32, tag="ones_f", name="ones_f")
        nc.vector.memset(ones_f[:], 1.0)
        ones1_f = const.tile([1, P], f32, tag="ones1_f", name="ones1_f")
        nc.vector.memset(ones1_f[:], 1.0)
        ones1_r = const.tile([1, P], f32r, tag="ones1_r", name="ones1_r")
        nc.vector.tensor_copy(ones1_r[:], ones1_f[:])
        onescol_r = const.tile([P, 1], f32r, tag="onescol_r", name="onescol_r")
        nc.vector.tensor_copy(onescol_r[:], ones_f[:, 0:1])

        def col_load(name, n):
            """1-D DRAM vector [n*128] -> sbuf [128, n] (partition-major)."""
            t = cols.tile([P, n], f32, tag=f"cols_{name}", name=f"cols_{name}")
            for j in range(n):
                nc.sync.dma_start(t[:, j:j + 1], ap[name][ds(P * j, P)])
            return t

        def bcast_rows(tag, row_f, n, psp, pool, rpool=None):
            """[1, n] fp32 row -> [128, n] fp32 tile via ones-matmul."""
            row_r = (rpool or pool).tile([1, n], f32r, tag=f"rr_{tag}",
                                         name=f"rr_{tag}")
            nc.vector.tensor_copy(row_r[:], row_f[0:1, 0:n])
            t = pool.tile([P, n], f32, tag=f"bc_{tag}", name=f"bc_{tag}")
            for h in range(0, n, GS):
                w = min(GS, n - h)
                pt = psp.tile([P, GS], f32, tag="bc_ps", name="bc_ps")
                nc.tensor.matmul(pt[:, 0:w], ones1_r[:], row_r[0:1, h:h + w],
                                 start=True, stop=True)
                nc.scalar.copy(t[:, h:h + w], pt[:, 0:w])
            return t

        bq_c = col_load("bq", DC)
        bk_c = col_load("bk", DC)
        Fb_c = col_load("Fb", KC)
        m1b_c = col_load("m1b", MC)

        bc4 = ctx.enter_context(tc.tile_pool(name="bc4", bufs=1))
        cs_row = rows.tile([1, 2 * K], f32, tag="cs", name="cs")

        # =========== phase A: conditioning vectors ===========
        with (
            tc.tile_pool(name="vec_sb", bufs=3) as vsb,
            tc.tile_pool(name="arow", bufs=1) as arow,
            tc.tile_pool(name="vec_ps", bufs=2, space="PSUM") as vps,
        ):
            def a_row_load(name, n):
                t = arow.tile([1, n], f32, tag=f"row_{name}", name=f"row_{name}")
                nc.sync.dma_start(t[:], ap[name][0:n])
                return t

            h1b_row = a_row_load("h1b", D)
            g1b_row = a_row_load("g1b", D)
            be1b_row = a_row_load("be1b", D)
            h2b_row = a_row_load("h2b", D)
            g2b_row = a_row_load("g2b", D)
            be2b_row = a_row_load("be2b", D)

            zc_f = cols.tile([P, DC], f32, tag="zc_f", name="zc_f")
            for j in range(DC):
                nc.sync.dma_start(zc_f[:, j:j + 1], ap["z"][0:1, ds(P * j, P)])
            zc_r = cols.tile([P, DC], f32r, tag="zc_r", name="zc_r")
            nc.vector.tensor_copy(zc_r[:], zc_f[:])

            def vec_layer(wname, lhs_cols, bias_row, act, out_row):
                """out_row[1, D] = act(lhs^T @ w + bias)."""
                pts = [vps.tile([1, GS], f32, tag=f"vps{h}", name=f"vps{h}")
                       for h in range(2)]
                for j in range(DC):
                    wt = vsb.tile([P, D], f32, tag="vw_f", name="vw_f")
                    nc.sync.dma_start(wt[:], ap[wname][ds(P * j, P), :])
                    wr = vsb.tile([P, D], f32r, tag="vw_r", name="vw_r")
                    nc.scalar.copy(wr[:], wt[:])
                    for h in range(2):
                        nc.tensor.matmul(pts[h][:], lhs_cols[:, j:j + 1],
                                         wr[:, ds(GS * h, GS)],
                                         start=(j == 0), stop=(j == DC - 1))
                for h in range(2):
                    pre = arow.tile([1, GS], f32, tag=f"vpre{h}", name=f"vpre{h}")
                    nc.vector.tensor_add(pre[:], pts[h][:],
                                         bias_row[0:1, ds(GS * h, GS)])
                    if act is None:
                        nc.vector.tensor_copy(out_row[0:1, ds(GS * h, GS)], pre[:])
                    else:
                        nc.scalar.activation(out_row[0:1, ds(GS * h, GS)],
                                             pre[:], act)

            def row_to_cols_r(tag, row_f):
                cf = cols.tile([P, DC], f32, tag=f"c_{tag}", name=f"c_{tag}")
                for j in range(DC):
                    nc.sync.dma_start(cf[:, j:j + 1], row_f[0:1, ds(P * j, P)])
                cr = cols.tile([P, DC], f32r, tag=f"cr_{tag}", name=f"cr_{tag}")
                nc.vector.tensor_copy(cr[:], cf[:])
                return cr

            h1_row = arow.tile([1, D], f32, tag="h1", name="h1")
            h2_row = arow.tile([1, D], f32, tag="h2", name="h2")
            sc1_row = arow.tile([1, D], f32, tag="sc1", name="sc1")
            of1_row = arow.tile([1, D], f32, tag="of1", name="of1")
            sc2_row = arow.tile([1, D], f32, tag="sc2", name="sc2")
            of2_row = arow.tile([1, D], f32, tag="of2", name="of2")
            vec_layer("h1w", zc_r, h1b_row, AF.Silu, h1_row)
            h1_c = row_to_cols_r("h1", h1_row)
            vec_layer("g1w", h1_c, g1b_row, None, sc1_row)
            vec_layer("be1w", h1_c, be1b_row, None, of1_row)
            vec_layer("h2w", zc_r, h2b_row, AF.Silu, h2_row)
            h2_c = row_to_cols_r("h2", h2_row)
            vec_layer("g2w", h2_c, g2b_row, None, sc2_row)
            vec_layer("be2w", h2_c, be2b_row, None, of2_row)
            scale1_b = bcast_rows("s1", sc1_row, D, vps, bc4, arow)
            offset1_b = bcast_rows("o1", of1_row, D, vps, bc4, arow)
            scale2_b = bcast_rows("s2", sc2_row, D, vps, bc4, arow)
            offset2_b = bcast_rows("o2", of2_row, D, vps, bc4, arow)

        # =========== phase W: fp32r DRAM copies of MLP weights ===========
        with tc.tile_pool(name="wr_sb", bufs=2) as wsb:
            for j in range(DC):
                t = wsb.tile([P, MLP], f32, tag="m1w_f", name="m1w_f")
                nc.sync.dma_start(t[:], ap["m1w"][ds(P * j, P), :])
                r = wsb.tile([P, MLP], f32r, tag="m1w_rr", name="m1w_rr")
                nc.scalar.copy(r[:], t[:])
                nc.sync.dma_start(m1w_r[ds(P * j, P), :], r[:])
            for m in range(MC):
                t = wsb.tile([P, D], f32, tag="m2w_f", name="m2w_f")
                nc.sync.dma_start(t[:], ap["m2w"][ds(P * m, P), :])
                r = wsb.tile([P, D], f32r, tag="m2w_rr", name="m2w_rr")
                nc.vector.tensor_copy(r[:], t[:])
                nc.sync.dma_start(m2w_r[ds(P * m, P), :], r[:])

        # manual pool stacks (LIFO per SBUF side)
        s_qT = contextlib.ExitStack()    # left
        s_x1n = contextlib.ExitStack()   # left
        s_kv = contextlib.ExitStack()    # left
        s_pef = contextlib.ExitStack()   # right
        s_aoT = contextlib.ExitStack()   # right
        s_x2T = contextlib.ExitStack()   # right
        try:
            # ===== phase B: adaLN1 + transposes + qT =====
            qT_p = s_qT.enter_context(tc.tile_pool(name="qT", bufs=1))
            qT = [[qT_p.tile([P, GS], f32r, tag=f"qT_{j}_{g}", name=f"qT_{j}_{g}")
                   for g in range(NG)] for j in range(DC)]
            x1n_p = s_x1n.enter_context(tc.tile_pool(name="x1nat", bufs=1))
            x1n = []
            if True:
                with (
                    tc.tile_pool(name="ln1_sb", bufs=2) as ln_sb,
                    tc.tile_pool(name="x1Trot", bufs=1) as x1t_p,
                    tc.tile_pool(name="wq_st", bufs=12) as wqst,
                    tc.tile_pool(name="tp1_ps", bufs=2, space="PSUM") as ln_ps,
                    tc.tile_pool(name="q_ps", bufs=3, space="PSUM") as qps,
                ):
                    for g in range(NG):
                        x1T_g = [x1t_p.tile([P, GS], f32r, tag=f"x1T_{j}",
                                            name=f"x1T_{j}") for j in range(DC)]
                        for ii in range(4):
                            i = 4 * g + ii
                            xt = ln_sb.tile([P, D], f32, tag="ln_in", name="ln_in")
                            nc.sync.dma_start(xt[:], ap["x"][ds(P * i, P), :])
                            st = ln_sb.tile([P, 2, 6], f32, tag="ln_st",
                                            name="ln_st")
                            nc.vector.bn_stats(st[:, 0, :], xt[:, 0:GS])
                            nc.vector.bn_stats(st[:, 1, :], xt[:, GS:D])
                            mv = ln_sb.tile([P, 2], f32, tag="ln_mv", name="ln_mv")
                            nc.vector.bn_aggr(mv[:], st[:])
                            sd = ln_sb.tile([P, 1], f32, tag="ln_sd", name="ln_sd")
                            nc.scalar.activation(sd[:], mv[:, 1:2], AF.Sqrt,
                                                 bias=eps_t[:])
                            rstd = ln_sb.tile([P, 1], f32, tag="ln_rstd",
                                              name="ln_rstd")
                            nc.vector.reciprocal(rstd[:], sd[:])
                            nc.vector.tensor_scalar(xt[:], xt[:], mv[:, 0:1],
                                                    rstd[:], OP.subtract, OP.mult)
                            nc.vector.tensor_mul(xt[:], xt[:], scale1_b[:])
                            x1t = x1n_p.tile([P, D], f32r, tag=f"nat{i}",
                                             name=f"nat{i}")
                            nc.vector.tensor_add(x1t[:], xt[:], offset1_b[:])
                            x1n.append(x1t)
                            for j in range(DC):
                                pt = ln_ps.tile([P, P], f32r, tag="tp_ps",
                                                name="tp_ps")
                                nc.tensor.transpose(pt[:], x1t[:, ds(P * j, P)],
                                                    ident_r[:])
                                nc.vector.tensor_copy(
                                    x1T_g[j][:, ds(P * ii, P)], pt[:])
                        for jo in range(DC):
                            pt = qps.tile([P, GS], f32, tag="q_ps", name="q_ps")
                            for j in range(DC):
                                wt = wqst.tile([P, P], f32, tag="wq_f",
                                               name="wq_f")
                                nc.sync.dma_start(
                                    wt[:], ap["wq"][ds(P * j, P), ds(P * jo, P)])
                                wr = wqst.tile([P, P], f32r, tag="wq_r",
                                               name="wq_r")
                                nc.scalar.copy(wr[:], wt[:])
                                nc.tensor.matmul(pt[:], wr[:], x1T_g[j][:],
                                                 start=(j == 0),
                                                 stop=(j == DC - 1))
                            nc.scalar.activation(qT[jo][g][:], pt[:], AF.Identity,
                                                 bias=bq_c[:, jo:jo + 1])

            # ===== phase B2: P_EF = x1^T @ [Ew|Fw] =====
            pef_sb = s_pef.enter_context(
                tc.tile_pool(name="pef_sb", bufs=1, side="right"))
            pef = [pef_sb.tile([P, 2 * K], f32r, tag=f"pefs{j}", name=f"pefs{j}")
                   for j in range(DC)]
            with (
                tc.tile_pool(name="ef_sb", bufs=3) as efsb,
                tc.tile_pool(name="pef_ps", bufs=1, space="PSUM") as pps,
            ):
                pef_ps = [pps.tile([P, 2 * K], f32, tag=f"pef{j}", name=f"pef{j}")
                          for j in range(DC)]
                for i in range(SC):
                    ef_f = efsb.tile([P, 2 * K], f32, tag="ef_f", name="ef_f")
                    nc.sync.dma_start(ef_f[:, 0:K], ap["Ew"][ds(P * i, P), :])
                    nc.sync.dma_start(ef_f[:, K:2 * K], ap["Fw"][ds(P * i, P), :])
                    ef_r = efsb.tile([P, 2 * K], f32r, tag="ef_r", name="ef_r")
                    nc.scalar.copy(ef_r[:], ef_f[:])
                    for j in range(DC):
                        nc.tensor.matmul(pef_ps[j][:], x1n[i][:, ds(P * j, P)],
                                         ef_r[:],
                                         start=(i == 0), stop=(i == SC - 1))
                for j in range(DC):
                    nc.scalar.copy(pef[j][:], pef_ps[j][:])
            s_x1n.close()

            # ===== phase KV: colsums, k_projT, v_proj_ext =====
            kv_sb = s_kv.enter_context(tc.tile_pool(name="kv_sb", bufs=1))
            kpT = [kv_sb.tile([P, K], f32r, tag=f"kpT{j}", name=f"kpT{j}")
                   for j in range(DC)]
            vpe = [kv_sb.tile([P, 65 * H], f32r, tag=f"vpe{c}", name=f"vpe{c}")
                   for c in range(KC)]
            with (
                tc.tile_pool(name="kv_st", bufs=3) as kvst,
                tc.tile_pool(name="kv_wv", bufs=1) as kvwv,
                tc.tile_pool(name="kv_tmp", bufs=2) as kvt,
                tc.tile_pool(name="kv_bias", bufs=1) as kvb,
                tc.tile_pool(name="cs_ps", bufs=1, space="PSUM") as csps,
                tc.tile_pool(name="kv_ps", bufs=2, space="PSUM") as kvps,
            ):
                cs_ps = csps.tile([1, 2 * K], f32, tag="cs_ps", name="cs_ps")
                for i in range(SC):
                    ef_f = kvst.tile([P, 2 * K], f32, tag="ef_f2", name="ef_f2")
                    nc.sync.dma_start(ef_f[:, 0:K], ap["Ew"][ds(P * i, P), :])
                    nc.sync.dma_start(ef_f[:, K:2 * K], ap["Fw"][ds(P * i, P), :])
                    ef_r = kvst.tile([P, 2 * K], f32r, tag="ef_r2", name="ef_r2")
                    nc.scalar.copy(ef_r[:], ef_f[:])
                    nc.tensor.matmul(cs_ps[:], onescol_r[:], ef_r[:],
                                     start=(i == 0), stop=(i == SC - 1))
                nc.vector.tensor_copy(cs_row[:], cs_ps[:])

                bv_row = kvb.tile([1, D], f32, tag="bv_row", name="bv_row")
                nc.sync.dma_start(bv_row[:], ap["bv"][0:D])
                Eb_row = kvb.tile([1, K], f32, tag="Eb_row", name="Eb_row")
                nc.sync.dma_start(Eb_row[:], ap["Eb"][0:K])
                bv_b = bcast_rows("bv", bv_row, D, kvps, kvb, kvt)
                Eb_b = bcast_rows("Eb", Eb_row, K, kvps, kvb, kvt)
                csE_b = bcast_rows("csE", cs_row, K, kvps, kvb, kvt)
                csF_c = kvb.tile([P, KC], f32, tag="csF_c", name="csF_c")
                for c in range(KC):
                    nc.sync.dma_start(csF_c[:, c:c + 1],
                                      cs_row[0:1, ds(K + P * c, P)])
                kp_bias = []
                for j in range(DC):
                    bt = kvb.tile([P, K], f32, tag=f"kpb{j}", name=f"kpb{j}")
                    nc.vector.tensor_scalar(bt[:], csE_b[:], bk_c[:, j:j + 1],
                                            None, OP.mult)
                    nc.vector.tensor_add(bt[:], bt[:], Eb_b[:])
                    kp_bias.append(bt)
                vp_bias = []
                for c in range(KC):
                    bt = kvb.tile([P, D], f32, tag=f"vpb{c}", name=f"vpb{c}")
                    nc.vector.tensor_scalar(bt[:], bv_b[:], csF_c[:, c:c + 1],
                                            Fb_c[:, c:c + 1], OP.mult, OP.add)
                    vp_bias.append(bt)

                for jo in range(DC):
                    pt = kvps.tile([P, K], f32, tag="kp_ps", name="kp_ps")
                    for j in range(DC):
                        wt = kvst.tile([P, P], f32, tag="wk_f", name="wk_f")
                        nc.sync.dma_start(
                            wt[:], ap["wk"][ds(P * j, P), ds(P * jo, P)])
                        wr = kvst.tile([P, P], f32r, tag="wk_r", name="wk_r")
                        nc.scalar.copy(wr[:], wt[:])
                        nc.tensor.matmul(pt[:], wr[:], pef[j][:, 0:K],
                                         start=(j == 0), stop=(j == DC - 1))
                    nc.vector.tensor_add(kpT[jo][:], pt[:], kp_bias[jo][:])
                for hf in range(2):
                    wvr = []
                    for j in range(DC):
                        wt = kvst.tile([P, GS], f32, tag="wv_f", name="wv_f")
                        nc.sync.dma_start(
                            wt[:], ap["wv"][ds(P * j, P), ds(GS * hf, GS)])
                        wr = kvwv.tile([P, GS], f32r, tag=f"wv_r{j}",
                                       name=f"wv_r{j}")
                        nc.scalar.copy(wr[:], wt[:])
                        wvr.append(wr)
                    for c in range(KC):
                        pt = kvps.tile([P, GS], f32, tag="vp_ps", name="vp_ps")
                        for j in range(DC):
                            nc.tensor.matmul(pt[:], pef[j][:, ds(K + P * c, P)],
                                             wvr[j][:],
                                             start=(j == 0), stop=(j == DC - 1))
                        tmp = kvt.tile([P, GS], f32, tag="vp_tmp", name="vp_tmp")
                        nc.vector.tensor_add(tmp[:], pt[:],
                                             vp_bias[c][:, ds(GS * hf, GS)])
                        for hh in range(8):
                            h = 8 * hf + hh
                            nc.vector.tensor_copy(vpe[c][:, ds(65 * h, 64)],
                                                  tmp[:, ds(64 * hh, 64)])
                for c in range(KC):
                    for h in range(H):
                        nc.vector.tensor_copy(vpe[c][:, ds(65 * h + 64, 1)],
                                              ones_f[:, 0:1])
            s_pef.close()

            # ===== phase C2: attention =====
            aoT_p = s_aoT.enter_context(
                tc.tile_pool(name="aoT", bufs=1, side="right"))
            aoT = [[aoT_p.tile([P, GS], f32r, tag=f"aoT_{j}_{g}",
                               name=f"aoT_{j}_{g}")
                    for g in range(NG)] for j in range(DC)]
            with (
                tc.tile_pool(name="at_sb", bufs=3) as atsb,
                tc.tile_pool(name="den_sb", bufs=2) as densb,
                tc.tile_pool(name="sc_ps", bufs=4, space="PSUM") as scps,
                tc.tile_pool(name="av_ps", bufs=2, space="PSUM") as avps,
                tc.tile_pool(name="bc2_ps", bufs=2, space="PSUM") as bcps,
            ):
                for g in range(NG):
                    for h in range(H):
                        j, r0 = h // 2, 64 * (h % 2)
                        exps = []
                        for c in range(KC):
                            spt = scps.tile([P, GS], f32, tag="sc", name="sc")
                            nc.tensor.matmul(spt[:],
                                             kpT[j][r0:r0 + 64, ds(P * c, P)],
                                             qT[j][g][r0:r0 + 64, :],
                                             start=True, stop=True)
                            et = atsb.tile([P, GS], f32r, tag="exp", name="exp")
                            nc.scalar.activation(et[:], spt[:], AF.Exp,
                                                 scale=0.125)
                            exps.append(et)
                        apt = avps.tile([65, GS], f32, tag="av", name="av")
                        for c in range(KC):
                            nc.tensor.matmul(apt[:], vpe[c][:, ds(65 * h, 65)],
                                             exps[c][:],
                                             start=(c == 0), stop=(c == KC - 1))
                        to = atsb.tile([64, GS], f32, tag="tmpo", name="tmpo")
                        nc.vector.tensor_copy(to[:], apt[0:64, :])
                        dh = densb.tile([1, GS], f32, tag="denh", name="denh")
                        nc.vector.tensor_copy(dh[:], apt[64:65, :])
                        rec = densb.tile([1, GS], f32, tag="rech", name="rech")
                        nc.vector.reciprocal(rec[:], dh[:])
                        rec_r = densb.tile([1, GS], f32r, tag="rech_r",
                                           name="rech_r")
                        nc.vector.tensor_copy(rec_r[:], rec[:])
                        bpt = bcps.tile([64, GS], f32, tag="bc2", name="bc2")
                        nc.tensor.matmul(bpt[:], ones1_r[0:1, 0:64], rec_r[:],
                                         start=True, stop=True)
                        bsb = atsb.tile([64, GS], f32, tag="bsb", name="bsb")
                        nc.scalar.copy(bsb[:], bpt[:])
                        nc.vector.tensor_mul(aoT[j][g][r0:r0 + 64, :],
                                             to[:], bsb[:])
            s_kv.close()
            s_qT.close()

            # ===== phase C3: wo + residual -> attn_sc =====
            with (
                tc.tile_pool(name="wo_sb", bufs=1) as wosb,
                tc.tile_pool(name="c3_sb", bufs=3) as c3sb,
                tc.tile_pool(name="c3_bc", bufs=1) as c3bc,
                tc.tile_pool(name="wo_ps", bufs=3, space="PSUM") as wops,
            ):
                with tc.tile_pool(name="c3_bc_ps", bufs=2, space="PSUM") as cbps:
                    bo_row = c3bc.tile([1, D], f32, tag="bo_row", name="bo_row")
                    nc.sync.dma_start(bo_row[:], ap["bo"][0:D])
                    bo_b = bcast_rows("bo", bo_row, D, cbps, c3bc, c3sb)
                wo_r = []
                for j in range(DC):
                    t = c3sb.tile([P, D], f32, tag="wo_f", name="wo_f")
                    nc.sync.dma_start(t[:], ap["wo"][ds(P * j, P), :])
                    r = wosb.tile([P, D], f32r, tag=f"wo_r{j}", name=f"wo_r{j}")
                    nc.scalar.copy(r[:], t[:])
                    wo_r.append(r)
                for i in range(SC):
                    g, c = i // 4, (i % 4) * P
                    xt = c3sb.tile([P, D], f32, tag="res_x", name="res_x")
                    nc.sync.dma_start(xt[:], ap["x"][ds(P * i, P), :])
                    at = c3sb.tile([P, D], f32, tag="attn_nat", name="attn_nat")
                    for hf in range(2):
                        pt = wops.tile([P, GS], f32, tag="wo_ps", name="wo_ps")
                        for j in range(DC):
                            nc.tensor.matmul(pt[:], aoT[j][g][:, ds(c, P)],
                                             wo_r[j][:, ds(GS * hf, GS)],
                                             start=(j == 0), stop=(j == DC - 1))
                        tm = c3sb.tile([P, GS], f32, tag="wo_tmp", name="wo_tmp")
                        nc.vector.tensor_add(tm[:], pt[:], bo_b[:, ds(GS * hf, GS)])
                        nc.vector.tensor_add(at[:, ds(GS * hf, GS)], tm[:],
                                             xt[:, ds(GS * hf, GS)])
                    nc.sync.dma_start(attn_sc[ds(P * i, P), :], at[:])
            s_aoT.close()

            # ===== phase D: adaLN2 + transpose =====
            x2T_p = s_x2T.enter_context(
                tc.tile_pool(name="x2T", bufs=1, side="right"))
            x2T = [[x2T_p.tile([P, GS], f32r, tag=f"x2T_{j}_{g}",
                               name=f"x2T_{j}_{g}")
                    for g in range(NG)] for j in range(DC)]
            with (
                tc.tile_pool(name="ln2_sb", bufs=2) as ln2_sb,
                tc.tile_pool(name="tp2_ps", bufs=2, space="PSUM") as ln2_ps,
            ):
                for i in range(SC):
                    g, ii = i // 4, i % 4
                    xt = ln2_sb.tile([P, D], f32, tag="ln2_in", name="ln2_in")
                    nc.sync.dma_start(xt[:], attn_sc[ds(P * i, P), :])
                    st = ln2_sb.tile([P, 2, 6], f32, tag="ln2_st", name="ln2_st")
                    nc.vector.bn_stats(st[:, 0, :], xt[:, 0:GS])
                    nc.vector.bn_stats(st[:, 1, :], xt[:, GS:D])
                    mv = ln2_sb.tile([P, 2], f32, tag="ln2_mv", name="ln2_mv")
                    nc.vector.bn_aggr(mv[:], st[:])
                    sd = ln2_sb.tile([P, 1], f32, tag="ln2_sd", name="ln2_sd")
                    nc.scalar.activation(sd[:], mv[:, 1:2], AF.Sqrt, bias=eps_t[:])
                    rstd = ln2_sb.tile([P, 1], f32, tag="ln2_rstd", name="ln2_rstd")
                    nc.vector.reciprocal(rstd[:], sd[:])
                    nc.vector.tensor_scalar(xt[:], xt[:], mv[:, 0:1], rstd[:],
                                            OP.subtract, OP.mult)
                    nc.vector.tensor_mul(xt[:], xt[:], scale2_b[:])
                    x2t = ln2_sb.tile([P, D], f32r, tag="x2nat", name="x2nat")
                    nc.vector.tensor_add(x2t[:], xt[:], offset2_b[:])
                    for j in range(DC):
                        pt = ln2_ps.tile([P, P], f32r, tag="tp2_ps", name="tp2_ps")
                        nc.tensor.transpose(pt[:], x2t[:, ds(P * j, P)], ident_r[:])
                        nc.vector.tensor_copy(x2T[j][g][:, ds(P * ii, P)], pt[:])

            # ===== phase E: MLP + residual -> out =====
            with tc.tile_pool(name="e_bc", bufs=1) as ebc:
                with tc.tile_pool(name="e_bc_ps", bufs=2, space="PSUM") as ebps:
                    m2b_row = ebc.tile([1, D], f32, tag="m2b_row", name="m2b_row")
                    nc.sync.dma_start(m2b_row[:], ap["m2b"][0:D])
                    m2b_b = bcast_rows("m2b", m2b_row, D, ebps, ebc)
                with (
                    tc.tile_pool(name="hmid", bufs=1) as hmid_p,
                    tc.tile_pool(name="mst", bufs=24) as mst,
                    tc.tile_pool(name="m2st", bufs=4) as m2st,
                    tc.tile_pool(name="e_sb", bufs=2) as esb,
                    tc.tile_pool(name="m1_ps", bufs=3, space="PSUM") as m1ps,
                    tc.tile_pool(name="m2_ps", bufs=1, space="PSUM") as m2ps,
                ):
                    for g in range(NG):
                        hm = [hmid_p.tile([P, GS], f32r, tag=f"hm{m}",
                                          name=f"hm{m}") for m in range(MC)]
                        for half in range(2):
                            m2p = [m2ps.tile([P, GS], f32, tag=f"m2p{ss}",
                                             name=f"m2p{ss}") for ss in range(4)]
                            for m in range(MC):
                                if half == 0:
                                    w1 = [mst.tile([P, P], f32r, tag="m1st",
                                                   name="m1st")
                                          for _ in range(DC)]
                                    for j in range(DC):
                                        nc.sync.dma_start(
                                            w1[j][:],
                                            m1w_r[ds(P * j, P), ds(P * m, P)])
                                    pt = m1ps.tile([P, GS], f32, tag="m1p",
                                                   name="m1p")
                                    for j in range(DC):
                                        nc.tensor.matmul(
                                            pt[:], w1[j][:], x2T[j][g][:],
                                            start=(j == 0), stop=(j == DC - 1))
                                    nc.scalar.activation(hm[m][:], pt[:], AF.Gelu,
                                                         bias=m1b_c[:, m:m + 1])
                                w2 = m2st.tile([P, GS], f32r, tag="m2stream",
                                               name="m2stream")
                                nc.sync.dma_start(
                                    w2[:], m2w_r[ds(P * m, P), ds(GS * half, GS)])
                                for ss in range(4):
                                    nc.tensor.matmul(
                                        m2p[ss][:], hm[m][:, ds(P * ss, P)], w2[:],
                                        start=(m == 0), stop=(m == MC - 1))
                            for ss in range(4):
                                i = 4 * g + ss
                                rt = esb.tile([P, GS], f32, tag="res_a",
                                              name="res_a")
                                nc.sync.dma_start(
                                    rt[:], attn_sc[ds(P * i, P), ds(GS * half, GS)])
                                tm = esb.tile([P, GS], f32, tag="e_tmp",
                                              name="e_tmp")
                                nc.vector.tensor_add(tm[:], m2p[ss][:],
                                                     m2b_b[:, ds(GS * half, GS)])
                                ot = esb.tile([P, GS], f32, tag="e_out",
                                              name="e_out")
                                nc.vector.tensor_add(ot[:], tm[:], rt[:])
                                nc.sync.dma_start(
                                    out[ds(P * i, P), ds(GS * half, GS)], ot[:])
            s_x2T.close()
        finally:
            for s in (s_x2T, s_aoT, s_pef, s_kv, s_qT, s_x1n):
                s.close()


def kernel(**inputs):
    nc = build()
    x = np.ascontiguousarray(inputs["x"], dtype=np.float32)
    z = np.ascontiguousarray(inputs["z"], dtype=np.float32)
    base = {}
    for nm, _, _ in W2D:
        base[nm] = np.ascontiguousarray(inputs[nm], dtype=np.float32)
    for nm, _ in W1D:
        base[nm] = np.ascontiguousarray(inputs[nm], dtype=np.float32)
    in_maps = []
    for c in range(B):
        m = dict(base)
        m["x"] = x[c]
        m["z"] = z[c:c + 1]
        in_maps.append(m)
    res = run_bass_kernel_spmd(nc, in_maps, list(range(B)))
    _cache["last"] = res
    return np.stack([res.results[c]["out"] for c in range(B)], axis=0)


# revision 11
# speedup vs baseline: 1.1270x; 1.1270x over previous
"""DiT block (Linformer attention + adaLN + MLP) on 8 TRN2 NeuronCores.

Sharding: data-parallel over batch (B=8 -> one batch element per core).
All matmuls run in float32r (full PE speed at free-dim>=256, ~1.6e-4 rel err).

Layout per core (S=2048 tokens, D=1024 features):
 - adaLN in natural [S_p, D_f] layout (bn_stats over free dim), PE-transpose
   x1 into x1T [D_p, S_f]; qT = wq^T @ x1T computed per token-group while
   the x1T tiles rotate.
 - Linformer K/V: k_projT = wk^T @ (x1^T @ Ew) -- the S->K projection is
   applied to x1 first (P_EF = x1^T @ [Ew|Fw]), so the wk/wv matmuls shrink
   by S/K = 8x and k,v are never materialized.
 - scores stay transposed: scoresT_h [K_p, S_f]; softmax denominators come
   from a fused ones-column appended to v_proj (65-row attn@V output);
   normalization uses a ones-matmul partition-broadcast of 1/denom.
 - MLP streams m1w/m2w from a pre-rounded fp32r DRAM copy in 4 token-groups.

SBUF pool lifetimes are split across the left/right allocator stacks so the
cross-phase handoff chain stays LIFO per side.
"""
import contextlib

import numpy as np

import concourse.bass as bass
import concourse.mybir as mybir
import concourse.tile as tile
from concourse import bacc
from concourse.bass import ds, ts
from concourse.bass_utils import run_bass_kernel_spmd
from concourse.masks import make_identity

f32 = mybir.dt.float32
f32r = mybir.dt.float32r
AF = mybir.ActivationFunctionType
OP = mybir.AluOpType

B, S, D, H, K, MLP, ZD = 8, 2048, 1024, 16, 256, 4096, 1024
DH = D // H      # 64
P = 128
SC = S // P      # 16 token chunks of 128
DC = D // P      # 8 feature chunks of 128
NG = 4           # token groups of 512
GS = 512
MC = MLP // P    # 32
KC = K // P      # 2
EPS = 1e-6

W2D = [("wq", D, D), ("wk", D, D), ("wv", D, D), ("wo", D, D),
       ("Ew", S, K), ("Fw", S, K),
       ("h1w", ZD, D), ("g1w", D, D), ("be1w", D, D),
       ("h2w", ZD, D), ("g2w", D, D), ("be2w", D, D),
       ("m1w", D, MLP), ("m2w", MLP, D)]
W1D = [("bq", D), ("bk", D), ("bv", D), ("bo", D), ("Eb", K), ("Fb", K),
       ("h1b", D), ("g1b", D), ("be1b", D), ("h2b", D), ("g2b", D), ("be2b", D),
       ("m1b", MLP), ("m2b", D)]

_cache = {}


def build():
    if "nc" in _cache:
        return _cache["nc"]
    nc = bacc.Bacc("TRN2", target_bir_lowering=False, debug=False, num_devices=8)
    ap = {}
    ap["x"] = nc.dram_tensor("x", [S, D], f32, kind="ExternalInput").ap()
    ap["z"] = nc.dram_tensor("z", [1, ZD], f32, kind="ExternalInput").ap()
    for nm, a, b in W2D:
        ap[nm] = nc.dram_tensor(nm, [a, b], f32, kind="ExternalInput").ap()
    for nm, a in W1D:
        ap[nm] = nc.dram_tensor(nm, [a], f32, kind="ExternalInput").ap()
    out = nc.dram_tensor("out", [S, D], f32, kind="ExternalOutput").ap()
    with tile.TileContext(nc, trace_sim=False) as tc:
        _emit(nc, tc, ap, out)
    nc.compile()
    _cache["nc"] = nc
    return nc


def _emit(nc, tc, ap, out):
    ctx = contextlib.ExitStack()
    with ctx:
        # ---------- whole-kernel pools ----------
        const = ctx.enter_context(tc.tile_pool(name="const", bufs=1))
        rows = ctx.enter_context(tc.tile_pool(name="rows", bufs=1))
        cols = ctx.enter_context(tc.tile_pool(name="cols", bufs=1))
        dram = ctx.enter_context(tc.tile_pool(name="dram", bufs=1, space="DRAM"))

        attn_sc = dram.tile([S, D], f32, tag="attn_sc", name="attn_sc")
        x1_sc = dram.tile([S, D], f32r, tag="x1_sc", name="x1_sc")

        ident_f = const.tile([P, P], f32, tag="ident_f", name="ident_f")
        make_identity(nc, ident_f)
        ident_r = const.tile([P, P], f32r, tag="ident_r", name="ident_r")
        nc.vector.tensor_copy(ident_r[:], ident_f[:])
        eps_t = const.tile([P, 1], f32, tag="eps", name="eps")
        nc.vector.memset(eps_t[:], EPS)
        ones_f = const.tile([P, 16], f32, tag="ones_f", name="ones_f")
        nc.vector.memset(ones_f[:], 1.0)
        ones1_f = const.tile([1, P], f32, tag="ones1_f", name="ones1_f")
        nc.vector.memset(ones1_f[:], 1.0)
        ones1_r = const.tile([1, P], f32r, tag="ones1_r", name="ones1_r")
        nc.vector.tensor_copy(ones1_r[:], ones1_f[:])
        onescol_r = const.tile([P, 1], f32r, tag="onescol_r", name="onescol_r")
        nc.vector.tensor_copy(onescol_r[:], ones_f[:, 0:1])

        def col_load(name, n):
            """1-D DRAM vector [n*128] -> sbuf [128, n] (partition-major)."""
            t = cols.tile([P, n], f32, tag=f"cols_{name}", name=f"cols_{name}")
            for j in range(n):
                nc.sync.dma_start(t[:, j:j + 1], ap[name][ds(P * j, P)])
            return t

        def bcast_rows(tag, row_f, n, psp, pool, rpool=None):
            """[1, n] fp32 row -> [128, n] fp32 tile via ones-matmul."""
            row_r = (rpool or pool).tile([1, n], f32r, tag=f"rr_{tag}",
                                         name=f"rr_{tag}")
            nc.vector.tensor_copy(row_r[:], row_f[0:1, 0:n])
            t = pool.tile([P, n], f32, tag=f"bc_{tag}", name=f"bc_{tag}")
            for h in range(0, n, GS):
                w = min(GS, n - h)
                pt = psp.tile([P, GS], f32, tag="bc_ps", name="bc_ps")
                nc.tensor.matmul(pt[:, 0:w], ones1_r[:], row_r[0:1, h:h + w],
                                 start=True, stop=True)
                nc.scalar.copy(t[:, h:h + w], pt[:, 0:w])
            return t

        bq_c = col_load("bq", DC)
        bk_c = col_load("bk", DC)
        Fb_c = col_load("Fb", KC)
        m1b_c = col_load("m1b", MC)

        bc4 = ctx.enter_context(tc.tile_pool(name="bc4", bufs=1))
        cs_row = rows.tile([1, 2 * K], f32, tag="cs", name="cs")

        # =========== phase A: conditioning vectors ===========
        with (
            tc.tile_pool(name="vec_sb", bufs=3) as vsb,
            tc.tile_pool(name="arow", bufs=1) as arow,
            tc.tile_pool(name="vec_ps", bufs=2, space="PSUM") as vps,
        ):
            def a_row_load(name, n):
                t = arow.tile([1, n], f32, tag=f"row_{name}", name=f"row_{name}")
                nc.sync.dma_start(t[:], ap[name][0:n])
                return t

            h1b_row = a_row_load("h1b", D)
            g1b_row = a_row_load("g1b", D)
            be1b_row = a_row_load("be1b", D)
            h2b_row = a_row_load("h2b", D)
            g2b_row = a_row_load("g2b", D)
            be2b_row = a_row_load("be2b", D)

            zc_f = cols.tile([P, DC], f32, tag="zc_f", name="zc_f")
            for j in range(DC):
                nc.sync.dma_start(zc_f[:, j:j + 1], ap["z"][0:1, ds(P * j, P)])
            zc_r = cols.tile([P, DC], f32r, tag="zc_r", name="zc_r")
            nc.vector.tensor_copy(zc_r[:], zc_f[:])

            def vec_layer(wname, lhs_cols, bias_row, act, out_row):
                """out_row[1, D] = act(lhs^T @ w + bias)."""
                pts = [vps.tile([1, GS], f32, tag=f"vps{h}", name=f"vps{h}")
                       for h in range(2)]
                for j in range(DC):
                    wt = vsb.tile([P, D], f32, tag="vw_f", name="vw_f")
                    nc.sync.dma_start(wt[:], ap[wname][ds(P * j, P), :])
                    wr = vsb.tile([P, D], f32r, tag="vw_r", name="vw_r")
                    nc.scalar.copy(wr[:], wt[:])
                    for h in range(2):
                        nc.tensor.matmul(pts[h][:], lhs_cols[:, j:j + 1],
                                         wr[:, ds(GS * h, GS)],
                                         start=(j == 0), stop=(j == DC - 1))
                for h in range(2):
                    pre = arow.tile([1, GS], f32, tag=f"vpre{h}", name=f"vpre{h}")
                    nc.vector.tensor_add(pre[:], pts[h][:],
                                         bias_row[0:1, ds(GS * h, GS)])
                    if act is None:
                        nc.vector.tensor_copy(out_row[0:1, ds(GS * h, GS)], pre[:])
                    else:
                        nc.scalar.activation(out_row[0:1, ds(GS * h, GS)],
                                             pre[:], act)

            def row_to_cols_r(tag, row_f):
                cf = cols.tile([P, DC], f32, tag=f"c_{tag}", name=f"c_{tag}")
                for j in range(DC):
                    nc.sync.dma_start(cf[:, j:j + 1], row_f[0:1, ds(P * j, P)])
                cr = cols.tile([P, DC], f32r, tag=f"cr_{tag}", name=f"cr_{tag}")
                nc.vector.tensor_copy(cr[:], cf[:])
                return cr

            h1_row = arow.tile([1, D], f32, tag="h1", name="h1")
            h2_row = arow.tile([1, D], f32, tag="h2", name="h2")
            sc1_row = arow.tile([1, D], f32, tag="sc1", name="sc1")
            of1_row = arow.tile([1, D], f32, tag="of1", name="of1")
            sc2_row = arow.tile([1, D], f32, tag="sc2", name="sc2")
            of2_row = arow.tile([1, D], f32, tag="of2", name="of2")
            vec_layer("h1w", zc_r, h1b_row, AF.Silu, h1_row)
            h1_c = row_to_cols_r("h1", h1_row)
            vec_layer("g1w", h1_c, g1b_row, None, sc1_row)
            vec_layer("be1w", h1_c, be1b_row, None, of1_row)
            vec_layer("h2w", zc_r, h2b_row, AF.Silu, h2_row)
            h2_c = row_to_cols_r("h2", h2_row)
            vec_layer("g2w", h2_c, g2b_row, None, sc2_row)
            vec_layer("be2w", h2_c, be2b_row, None, of2_row)
            scale1_b = bcast_rows("s1", sc1_row, D, vps, bc4, arow)
            offset1_b = bcast_rows("o1", of1_row, D, vps, bc4, arow)
            scale2_b = bcast_rows("s2", sc2_row, D, vps, bc4, arow)
            offset2_b = bcast_rows("o2", of2_row, D, vps, bc4, arow)

        # manual pool stacks (LIFO per SBUF side)
        s_qT = contextlib.ExitStack()    # left
        s_kv = contextlib.ExitStack()    # left
        s_pef = contextlib.ExitStack()   # right
        s_aoT = contextlib.ExitStack()   # right
        try:
            # ===== phase B: adaLN1 + transposes + qT =====
            qT_p = s_qT.enter_context(tc.tile_pool(name="qT", bufs=1))
            qT = [[qT_p.tile([P, GS], f32r, tag=f"qT_{j}_{g}", name=f"qT_{j}_{g}")
                   for g in range(NG)] for j in range(DC)]
            x1n = []
            if True:
                with (
                    tc.tile_pool(name="ln1_sb", bufs=2) as ln_sb,
                    tc.tile_pool(name="x1Trot", bufs=2) as x1t_p,
                    tc.tile_pool(name="wq_st", bufs=12) as wqst,
                    tc.tile_pool(name="tp1_ps", bufs=2, space="PSUM") as ln_ps,
                    tc.tile_pool(name="q_ps", bufs=3, space="PSUM") as qps,
                ):
                    for g in range(NG):
                        x1T_g = [x1t_p.tile([P, GS], f32r, tag=f"x1T_{j}",
                                            name=f"x1T_{j}") for j in range(DC)]
                        for ii in range(4):
                            i = 4 * g + ii
                            xt = ln_sb.tile([P, D], f32, tag="ln_in", name="ln_in")
                            nc.sync.dma_start(xt[:], ap["x"][ds(P * i, P), :])
                            st = ln_sb.tile([P, 2, 6], f32, tag="ln_st",
                                            name="ln_st")
                            nc.vector.bn_stats(st[:, 0, :], xt[:, 0:GS])
                            nc.vector.bn_stats(st[:, 1, :], xt[:, GS:D])
                            mv = ln_sb.tile([P, 2], f32, tag="ln_mv", name="ln_mv")
                            nc.vector.bn_aggr(mv[:], st[:])
                            sd = ln_sb.tile([P, 1], f32, tag="ln_sd", name="ln_sd")
                            nc.scalar.activation(sd[:], mv[:, 1:2], AF.Sqrt,
                                                 bias=eps_t[:])
                            rstd = ln_sb.tile([P, 1], f32, tag="ln_rstd",
                                              name="ln_rstd")
                            nc.vector.reciprocal(rstd[:], sd[:])
                            nc.vector.tensor_scalar(xt[:], xt[:], mv[:, 0:1],
                                                    rstd[:], OP.subtract, OP.mult)
                            nc.vector.tensor_mul(xt[:], xt[:], scale1_b[:])
                            x1t = ln_sb.tile([P, D], f32r, tag="nat",
                                             name="nat")
                            nc.vector.tensor_add(x1t[:], xt[:], offset1_b[:])
                            nc.sync.dma_start(x1_sc[ds(P * i, P), :], x1t[:])
                            for j in range(DC):
                                pt = ln_ps.tile([P, P], f32r, tag="tp_ps",
                                                name="tp_ps")
                                nc.tensor.transpose(pt[:], x1t[:, ds(P * j, P)],
                                                    ident_r[:])
                                nc.vector.tensor_copy(
                                    x1T_g[j][:, ds(P * ii, P)], pt[:])
                        for jo in range(DC):
                            pt = qps.tile([P, GS], f32, tag="q_ps", name="q_ps")
                            for j in range(DC):
                                wt = wqst.tile([P, P], f32, tag="wq_f",
                                               name="wq_f")
                                nc.sync.dma_start(
                                    wt[:], ap["wq"][ds(P * j, P), ds(P * jo, P)])
                                wr = wqst.tile([P, P], f32r, tag="wq_r",
                                               name="wq_r")
                                nc.scalar.copy(wr[:], wt[:])
                                nc.tensor.matmul(pt[:], wr[:], x1T_g[j][:],
                                                 start=(j == 0),
                                                 stop=(j == DC - 1))
                            nc.scalar.activation(qT[jo][g][:], pt[:], AF.Identity,
                                                 bias=bq_c[:, jo:jo + 1])

            # ===== phase B2: P_EF = x1^T @ [Ew|Fw] =====
            pef_sb = s_pef.enter_context(
                tc.tile_pool(name="pef_sb", bufs=1, side="right"))
            pef = [pef_sb.tile([P, 2 * K], f32r, tag=f"pefs{j}", name=f"pefs{j}")
                   for j in range(DC)]
            with (
                tc.tile_pool(name="ef_sb", bufs=3) as efsb,
                tc.tile_pool(name="pef_ps", bufs=1, space="PSUM") as pps,
            ):
                pef_ps = [pps.tile([P, 2 * K], f32, tag=f"pef{j}", name=f"pef{j}")
                          for j in range(DC)]
                for i in range(SC):
                    xn_t = efsb.tile([P, D], f32r, tag="x1s", name="x1s")
                    nc.sync.dma_start(xn_t[:], x1_sc[ds(P * i, P), :])
                    ef_f = efsb.tile([P, 2 * K], f32, tag="ef_f", name="ef_f")
                    nc.sync.dma_start(ef_f[:, 0:K], ap["Ew"][ds(P * i, P), :])
                    nc.sync.dma_start(ef_f[:, K:2 * K], ap["Fw"][ds(P * i, P), :])
                    ef_r = efsb.tile([P, 2 * K], f32r, tag="ef_r", name="ef_r")
                    nc.scalar.copy(ef_r[:], ef_f[:])
                    for j in range(DC):
                        nc.tensor.matmul(pef_ps[j][:], xn_t[:, ds(P * j, P)],
                                         ef_r[:],
                                         start=(i == 0), stop=(i == SC - 1))
                for j in range(DC):
                    nc.scalar.copy(pef[j][:], pef_ps[j][:])

            # ===== phase KV: colsums, k_projT, v_proj_ext =====
            kv_sb = s_kv.enter_context(tc.tile_pool(name="kv_sb", bufs=1))
            kpT = [kv_sb.tile([P, K], f32r, tag=f"kpT{j}", name=f"kpT{j}")
                   for j in range(DC)]
            vpe = [kv_sb.tile([P, 65 * H], f32r, tag=f"vpe{c}", name=f"vpe{c}")
                   for c in range(KC)]
            with (
                tc.tile_pool(name="kv_st", bufs=3) as kvst,
                tc.tile_pool(name="kv_wv", bufs=1) as kvwv,
                tc.tile_pool(name="kv_tmp", bufs=2) as kvt,
                tc.tile_pool(name="kv_bias", bufs=1) as kvb,
                tc.tile_pool(name="cs_ps", bufs=1, space="PSUM") as csps,
                tc.tile_pool(name="kv_ps", bufs=2, space="PSUM") as kvps,
            ):
                cs_ps = csps.tile([1, 2 * K], f32, tag="cs_ps", name="cs_ps")
                for i in range(SC):
                    ef_f = kvst.tile([P, 2 * K], f32, tag="ef_f2", name="ef_f2")
                    nc.sync.dma_start(ef_f[:, 0:K], ap["Ew"][ds(P * i, P), :])
                    nc.sync.dma_start(ef_f[:, K:2 * K], ap["Fw"][ds(P * i, P), :])
                    ef_r = kvst.tile([P, 2 * K], f32r, tag="ef_r2", name="ef_r2")
                    nc.scalar.copy(ef_r[:], ef_f[:])
                    nc.tensor.matmul(cs_ps[:], onescol_r[:], ef_r[:],
                                     start=(i == 0), stop=(i == SC - 1))
                nc.vector.tensor_copy(cs_row[:], cs_ps[:])

                bv_row = kvb.tile([1, D], f32, tag="bv_row", name="bv_row")
                nc.sync.dma_start(bv_row[:], ap["bv"][0:D])
                Eb_row = kvb.tile([1, K], f32, tag="Eb_row", name="Eb_row")
                nc.sync.dma_start(Eb_row[:], ap["Eb"][0:K])
                bv_b = bcast_rows("bv", bv_row, D, kvps, kvb, kvt)
                Eb_b = bcast_rows("Eb", Eb_row, K, kvps, kvb, kvt)
                csE_b = bcast_rows("csE", cs_row, K, kvps, kvb, kvt)
                csF_c = kvb.tile([P, KC], f32, tag="csF_c", name="csF_c")
                for c in range(KC):
                    nc.sync.dma_start(csF_c[:, c:c + 1],
                                      cs_row[0:1, ds(K + P * c, P)])
                kp_bias = []
                for j in range(DC):
                    bt = kvb.tile([P, K], f32, tag=f"kpb{j}", name=f"kpb{j}")
                    nc.vector.tensor_scalar(bt[:], csE_b[:], bk_c[:, j:j + 1],
                                            None, OP.mult)
                    nc.vector.tensor_add(bt[:], bt[:], Eb_b[:])
                    kp_bias.append(bt)
                vp_bias = []
                for c in range(KC):
                    bt = kvb.tile([P, D], f32, tag=f"vpb{c}", name=f"vpb{c}")
                    nc.vector.tensor_scalar(bt[:], bv_b[:], csF_c[:, c:c + 1],
                                            Fb_c[:, c:c + 1], OP.mult, OP.add)
                    vp_bias.append(bt)

                for jo in range(DC):
                    pt = kvps.tile([P, K], f32, tag="kp_ps", name="kp_ps")
                    for j in range(DC):
                        wt = kvst.tile([P, P], f32, tag="wk_f", name="wk_f")
                        nc.sync.dma_start(
                            wt[:], ap["wk"][ds(P * j, P), ds(P * jo, P)])
                        wr = kvst.tile([P, P], f32r, tag="wk_r", name="wk_r")
                        nc.scalar.copy(wr[:], wt[:])
                        nc.tensor.matmul(pt[:], wr[:], pef[j][:, 0:K],
                                         start=(j == 0), stop=(j == DC - 1))
                    nc.vector.tensor_add(kpT[jo][:], pt[:], kp_bias[jo][:])
                for hf in range(2):
                    wvr = []
                    for j in range(DC):
                        wt = kvst.tile([P, GS], f32, tag="wv_f", name="wv_f")
                        nc.sync.dma_start(
                            wt[:], ap["wv"][ds(P * j, P), ds(GS * hf, GS)])
                        wr = kvwv.tile([P, GS], f32r, tag=f"wv_r{j}",
                                       name=f"wv_r{j}")
                        nc.scalar.copy(wr[:], wt[:])
                        wvr.append(wr)
                    for c in range(KC):
                        pt = kvps.tile([P, GS], f32, tag="vp_ps", name="vp_ps")
                        for j in range(DC):
                            nc.tensor.matmul(pt[:], pef[j][:, ds(K + P * c, P)],
                                             wvr[j][:],
                                             start=(j == 0), stop=(j == DC - 1))
                        tmp = kvt.tile([P, GS], f32, tag="vp_tmp", name="vp_tmp")
                        nc.vector.tensor_add(tmp[:], pt[:],
                                             vp_bias[c][:, ds(GS * hf, GS)])
                        for hh in range(8):
                            h = 8 * hf + hh
                            nc.vector.tensor_copy(vpe[c][:, ds(65 * h, 64)],
                                                  tmp[:, ds(64 * hh, 64)])
                for c in range(KC):
                    for h in range(H):
                        nc.vector.tensor_copy(vpe[c][:, ds(65 * h + 64, 1)],
                                              ones_f[:, 0:1])
            s_pef.close()

            # ===== phase C2: attention =====
            aoT_p = s_aoT.enter_context(
                tc.tile_pool(name="aoT", bufs=1, side="right"))
            aoT = [[aoT_p.tile([P, GS], f32r, tag=f"aoT_{j}_{g}",
                               name=f"aoT_{j}_{g}")
                    for g in range(NG)] for j in range(DC)]
            with (
                tc.tile_pool(name="at_sb", bufs=3) as atsb,
                tc.tile_pool(name="den_sb", bufs=2) as densb,
                tc.tile_pool(name="sc_ps", bufs=4, space="PSUM") as scps,
                tc.tile_pool(name="av_ps", bufs=2, space="PSUM") as avps,
                tc.tile_pool(name="bc2_ps", bufs=2, space="PSUM") as bcps,
            ):
                def nrm_flush(p):
                    to, rec_r, j, r0, g = p
                    bpt = bcps.tile([64, GS], f32, tag="bc2", name="bc2")
                    nc.tensor.matmul(bpt[:], ones1_r[0:1, 0:64], rec_r[:],
                                     start=True, stop=True)
                    bsb = atsb.tile([64, GS], f32, tag="bsb", name="bsb")
                    nc.scalar.copy(bsb[:], bpt[:])
                    nc.vector.tensor_mul(aoT[j][g][r0:r0 + 64, :], to[:], bsb[:])

                for g in range(NG):
                    pend = None
                    for h in range(H):
                        j, r0 = h // 2, 64 * (h % 2)
                        exps = []
                        for c in range(KC):
                            spt = scps.tile([P, GS], f32, tag="sc", name="sc")
                            nc.tensor.matmul(spt[:],
                                             kpT[j][r0:r0 + 64, ds(P * c, P)],
                                             qT[j][g][r0:r0 + 64, :],
                                             start=True, stop=True)
                            et = atsb.tile([P, GS], f32r, tag="exp", name="exp")
                            nc.scalar.activation(et[:], spt[:], AF.Exp,
                                                 scale=0.125)
                            exps.append(et)
                        apt = avps.tile([65, GS], f32, tag="av", name="av")
                        for c in range(KC):
                            nc.tensor.matmul(apt[:], vpe[c][:, ds(65 * h, 65)],
                                             exps[c][:],
                                             start=(c == 0), stop=(c == KC - 1))
                        # normalize the PREVIOUS head now -- its reciprocal is
                        # ready, so the broadcast matmul doesn't stall the PE
                        if pend is not None:
                            nrm_flush(pend)
                        to = atsb.tile([64, GS], f32, tag="tmpo", name="tmpo")
                        nc.vector.tensor_copy(to[:], apt[0:64, :])
                        dh = densb.tile([1, GS], f32, tag="denh", name="denh")
                        nc.vector.tensor_copy(dh[:], apt[64:65, :])
                        rec = densb.tile([1, GS], f32, tag="rech", name="rech")
                        nc.vector.reciprocal(rec[:], dh[:])
                        rec_r = densb.tile([1, GS], f32r, tag="rech_r",
                                           name="rech_r")
                        nc.vector.tensor_copy(rec_r[:], rec[:])
                        pend = (to, rec_r, j, r0, g)
                    nrm_flush(pend)
            s_kv.close()
            s_qT.close()

            # ===== phase C3: wo + residual -> attn_sc =====
            with (
                tc.tile_pool(name="wo_sb", bufs=1) as wosb,
                tc.tile_pool(name="c3_sb", bufs=3) as c3sb,
                tc.tile_pool(name="c3_bc", bufs=1) as c3bc,
                tc.tile_pool(name="wo_ps", bufs=3, space="PSUM") as wops,
            ):
                with tc.tile_pool(name="c3_bc_ps", bufs=2, space="PSUM") as cbps:
                    bo_row = c3bc.tile([1, D], f32, tag="bo_row", name="bo_row")
                    nc.sync.dma_start(bo_row[:], ap["bo"][0:D])
                    bo_b = bcast_rows("bo", bo_row, D, cbps, c3bc, c3sb)
                wo_r = []
                for j in range(DC):
                    t = c3sb.tile([P, D], f32, tag="wo_f", name="wo_f")
                    nc.sync.dma_start(t[:], ap["wo"][ds(P * j, P), :])
                    r = wosb.tile([P, D], f32r, tag=f"wo_r{j}", name=f"wo_r{j}")
                    nc.scalar.copy(r[:], t[:])
                    wo_r.append(r)
                for i in range(SC):
                    g, c = i // 4, (i % 4) * P
                    xt = c3sb.tile([P, D], f32, tag="res_x", name="res_x")
                    nc.sync.dma_start(xt[:], ap["x"][ds(P * i, P), :])
                    at = c3sb.tile([P, D], f32, tag="attn_nat", name="attn_nat")
                    for hf in range(2):
                        pt = wops.tile([P, GS], f32, tag="wo_ps", name="wo_ps")
                        for j in range(DC):
                            nc.tensor.matmul(pt[:], aoT[j][g][:, ds(c, P)],
                                             wo_r[j][:, ds(GS * hf, GS)],
                                             start=(j == 0), stop=(j == DC - 1))
                        tm = c3sb.tile([P, GS], f32, tag="wo_tmp", name="wo_tmp")
                        nc.vector.tensor_add(tm[:], pt[:], bo_b[:, ds(GS * hf, GS)])
                        nc.vector.tensor_add(at[:, ds(GS * hf, GS)], tm[:],
                                             xt[:, ds(GS * hf, GS)])
                    nc.sync.dma_start(attn_sc[ds(P * i, P), :], at[:])
            s_aoT.close()

            # ===== phase D+E: adaLN2 -> MLP, fused per token-group =====
            with tc.tile_pool(name="e_bc", bufs=1) as ebc:
                with tc.tile_pool(name="e_bc_ps", bufs=2, space="PSUM") as ebps:
                    m2b_row = ebc.tile([1, D], f32, tag="m2b_row", name="m2b_row")
                    nc.sync.dma_start(m2b_row[:], ap["m2b"][0:D])
                    m2b_b = bcast_rows("m2b", m2b_row, D, ebps, ebc)
                with (
                    tc.tile_pool(name="ln2_sb", bufs=2) as ln2_sb,
                    tc.tile_pool(name="x2Trot", bufs=2) as x2t_p,
                    tc.tile_pool(name="hmid", bufs=1) as hmid_p,
                    tc.tile_pool(name="mst", bufs=24) as mst,
                    tc.tile_pool(name="m2st", bufs=4) as m2st,
                    tc.tile_pool(name="e_sb", bufs=2) as esb,
                    tc.tile_pool(name="tp2_ps", bufs=2, space="PSUM") as ln2_ps,
                    tc.tile_pool(name="m1_ps", bufs=2, space="PSUM") as m1ps,
                    tc.tile_pool(name="m2_ps", bufs=1, space="PSUM") as m2ps,
                ):
                    for g in range(NG):
                        x2T_g = [x2t_p.tile([P, GS], f32r, tag=f"x2T_{j}",
                                            name=f"x2T_{j}") for j in range(DC)]
                        for ii in range(4):
                            i = 4 * g + ii
                            xt = ln2_sb.tile([P, D], f32, tag="ln2_in",
                                             name="ln2_in")
                            nc.sync.dma_start(xt[:], attn_sc[ds(P * i, P), :])
                            st = ln2_sb.tile([P, 2, 6], f32, tag="ln2_st",
                                             name="ln2_st")
                            nc.vector.bn_stats(st[:, 0, :], xt[:, 0:GS])
                            nc.vector.bn_stats(st[:, 1, :], xt[:, GS:D])
                            mv = ln2_sb.tile([P, 2], f32, tag="ln2_mv",
                                             name="ln2_mv")
                            nc.vector.bn_aggr(mv[:], st[:])
                            sd = ln2_sb.tile([P, 1], f32, tag="ln2_sd",
                                             name="ln2_sd")
                            nc.scalar.activation(sd[:], mv[:, 1:2], AF.Sqrt,
                                                 bias=eps_t[:])
                            rstd = ln2_sb.tile([P, 1], f32, tag="ln2_rstd",
                                               name="ln2_rstd")
                            nc.vector.reciprocal(rstd[:], sd[:])
                            nc.vector.tensor_scalar(xt[:], xt[:], mv[:, 0:1],
                                                    rstd[:], OP.subtract, OP.mult)
                            nc.vector.tensor_mul(xt[:], xt[:], scale2_b[:])
                            x2t = ln2_sb.tile([P, D], f32r, tag="x2nat",
                                              name="x2nat")
                            nc.vector.tensor_add(x2t[:], xt[:], offset2_b[:])
                            for j in range(DC):
                                pt = ln2_ps.tile([P, P], f32r, tag="tp2_ps",
                                                 name="tp2_ps")
                                nc.tensor.transpose(pt[:], x2t[:, ds(P * j, P)],
                                                    ident_r[:])
                                nc.vector.tensor_copy(
                                    x2T_g[j][:, ds(P * ii, P)], pt[:])
                        hm = [hmid_p.tile([P, GS], f32r, tag=f"hm{m}",
                                          name=f"hm{m}") for m in range(MC)]
                        for half in range(2):
                            m2p = [m2ps.tile([P, GS], f32, tag=f"m2p{ss}",
                                             name=f"m2p{ss}") for ss in range(4)]
                            for m in range(MC):
                                if half == 0:
                                    w1f = [mst.tile([P, P], f32, tag="m1st",
                                                    name="m1st")
                                           for _ in range(DC)]
                                    w1r = [mst.tile([P, P], f32r, tag="m1str",
                                                    name="m1str")
                                           for _ in range(DC)]
                                    for j in range(DC):
                                        nc.sync.dma_start(
                                            w1f[j][:],
                                            ap["m1w"][ds(P * j, P), ds(P * m, P)])
                                        nc.scalar.copy(w1r[j][:], w1f[j][:])
                                    pt = m1ps.tile([P, GS], f32, tag="m1p",
                                                   name="m1p")
                                    for j in range(DC):
                                        nc.tensor.matmul(
                                            pt[:], w1r[j][:], x2T_g[j][:],
                                            start=(j == 0), stop=(j == DC - 1))
                                    nc.scalar.activation(hm[m][:], pt[:], AF.Gelu,
                                                         bias=m1b_c[:, m:m + 1])
                                w2f = m2st.tile([P, GS], f32, tag="m2f",
                                                name="m2f")
                                nc.sync.dma_start(
                                    w2f[:], ap["m2w"][ds(P * m, P),
                                                      ds(GS * half, GS)])
                                w2r = m2st.tile([P, GS], f32r, tag="m2r",
                                                name="m2r")
                                nc.scalar.copy(w2r[:], w2f[:])
                                for ss in range(4):
                                    nc.tensor.matmul(
                                        m2p[ss][:], hm[m][:, ds(P * ss, P)],
                                        w2r[:],
                                        start=(m == 0), stop=(m == MC - 1))
                            for ss in range(4):
                                i = 4 * g + ss
                                rt = esb.tile([P, GS], f32, tag="res_a",
                                              name="res_a")
                                nc.sync.dma_start(
                                    rt[:], attn_sc[ds(P * i, P),
                                                   ds(GS * half, GS)])
                                tm = esb.tile([P, GS], f32, tag="e_tmp",
                                              name="e_tmp")
                                nc.vector.tensor_add(tm[:], m2p[ss][:],
                                                     m2b_b[:, ds(GS * half, GS)])
                                ot = esb.tile([P, GS], f32, tag="e_out",
                                              name="e_out")
                                nc.vector.tensor_add(ot[:], tm[:], rt[:])
                                nc.sync.dma_start(
                                    out[ds(P * i, P), ds(GS * half, GS)], ot[:])
        finally:
            for s in (s_aoT, s_pef, s_kv, s_qT):
                s.close()


def kernel(**inputs):
    nc = build()
    x = np.ascontiguousarray(inputs["x"], dtype=np.float32)
    z = np.ascontiguousarray(inputs["z"], dtype=np.float32)
    base = {}
    for nm, _, _ in W2D:
        base[nm] = np.ascontiguousarray(inputs[nm], dtype=np.float32)
    for nm, _ in W1D:
        base[nm] = np.ascontiguousarray(inputs[nm], dtype=np.float32)
    in_maps = []
    for c in range(B):
        m = dict(base)
        m["x"] = x[c]
        m["z"] = z[c:c + 1]
        in_maps.append(m)
    res = run_bass_kernel_spmd(nc, in_maps, list(range(B)))
    _cache["last"] = res
    return np.stack([res.results[c]["out"] for c in range(B)], axis=0)


# revision 12
# speedup vs baseline: 1.1337x; 1.0059x over previous
"""DiT block (Linformer attention + adaLN + MLP) on 8 TRN2 NeuronCores.

Sharding: data-parallel over batch (B=8 -> one batch element per core).
All matmuls run in float32r (full PE speed at free-dim>=256, ~1.6e-4 rel err).

Layout per core (S=2048 tokens, D=1024 features):
 - adaLN in natural [S_p, D_f] layout (bn_stats over free dim), PE-transpose
   x1 into x1T [D_p, S_f]; qT = wq^T @ x1T computed per token-group while
   the x1T tiles rotate.
 - Linformer K/V: k_projT = wk^T @ (x1^T @ Ew) -- the S->K projection is
   applied to x1 first (P_EF = x1^T @ [Ew|Fw]), so the wk/wv matmuls shrink
   by S/K = 8x and k,v are never materialized.
 - scores stay transposed: scoresT_h [K_p, S_f]; softmax denominators come
   from a fused ones-column appended to v_proj (65-row attn@V output);
   normalization uses a ones-matmul partition-broadcast of 1/denom.
 - MLP streams m1w/m2w from a pre-rounded fp32r DRAM copy in 4 token-groups.

SBUF pool lifetimes are split across the left/right allocator stacks so the
cross-phase handoff chain stays LIFO per side.
"""
import contextlib

import numpy as np

import concourse.bass as bass
import concourse.mybir as mybir
import concourse.tile as tile
from concourse import bacc
from concourse.bass import ds, ts
from concourse.bass_utils import run_bass_kernel_spmd
from concourse.masks import make_identity

f32 = mybir.dt.float32
f32r = mybir.dt.float32r
f16 = mybir.dt.float16
AF = mybir.ActivationFunctionType
OP = mybir.AluOpType

B, S, D, H, K, MLP, ZD = 8, 2048, 1024, 16, 256, 4096, 1024
DH = D // H      # 64
P = 128
SC = S // P      # 16 token chunks of 128
DC = D // P      # 8 feature chunks of 128
NG = 4           # token groups of 512
GS = 512
MC = MLP // P    # 32
KC = K // P      # 2
EPS = 1e-6

W2D = [("wq", D, D), ("wk", D, D), ("wv", D, D), ("wo", D, D),
       ("Ew", S, K), ("Fw", S, K),
       ("h1w", ZD, D), ("g1w", D, D), ("be1w", D, D),
       ("h2w", ZD, D), ("g2w", D, D), ("be2w", D, D),
       ("m1w", D, MLP), ("m2w", MLP, D)]
W1D = [("bq", D), ("bk", D), ("bv", D), ("bo", D), ("Eb", K), ("Fb", K),
       ("h1b", D), ("g1b", D), ("be1b", D), ("h2b", D), ("g2b", D), ("be2b", D),
       ("m1b", MLP), ("m2b", D)]

_cache = {}


def build():
    if "nc" in _cache:
        return _cache["nc"]
    nc = bacc.Bacc("TRN2", target_bir_lowering=False, debug=False, num_devices=8)
    ap = {}
    ap["x"] = nc.dram_tensor("x", [S, D], f32, kind="ExternalInput").ap()
    ap["z"] = nc.dram_tensor("z", [1, ZD], f32, kind="ExternalInput").ap()
    for nm, a, b in W2D:
        ap[nm] = nc.dram_tensor(nm, [a, b], f32, kind="ExternalInput").ap()
    for nm, a in W1D:
        ap[nm] = nc.dram_tensor(nm, [a], f32, kind="ExternalInput").ap()
    out = nc.dram_tensor("out", [S, D], f32, kind="ExternalOutput").ap()
    with tile.TileContext(nc, trace_sim=False) as tc:
        _emit(nc, tc, ap, out)
    nc.compile()
    _cache["nc"] = nc
    return nc


def _emit(nc, tc, ap, out):
    ctx = contextlib.ExitStack()
    with ctx:
        # ---------- whole-kernel pools ----------
        const = ctx.enter_context(tc.tile_pool(name="const", bufs=1))
        rows = ctx.enter_context(tc.tile_pool(name="rows", bufs=1))
        cols = ctx.enter_context(tc.tile_pool(name="cols", bufs=1))
        dram = ctx.enter_context(tc.tile_pool(name="dram", bufs=1, space="DRAM"))

        attn_sc = dram.tile([S, D], f32, tag="attn_sc", name="attn_sc")
        x1_sc = dram.tile([S, D], f32r, tag="x1_sc", name="x1_sc")

        ident_f = const.tile([P, P], f32, tag="ident_f", name="ident_f")
        make_identity(nc, ident_f)
        ident_r = const.tile([P, P], f32r, tag="ident_r", name="ident_r")
        nc.vector.tensor_copy(ident_r[:], ident_f[:])
        eps_t = const.tile([P, 1], f32, tag="eps", name="eps")
        nc.vector.memset(eps_t[:], EPS)
        ones_f = const.tile([P, 16], f32, tag="ones_f", name="ones_f")
        nc.vector.memset(ones_f[:], 1.0)
        ones1_f = const.tile([1, P], f32, tag="ones1_f", name="ones1_f")
        nc.vector.memset(ones1_f[:], 1.0)
        ones1_r = const.tile([1, P], f32r, tag="ones1_r", name="ones1_r")
        nc.vector.tensor_copy(ones1_r[:], ones1_f[:])
        onescol_r = const.tile([P, 1], f32r, tag="onescol_r", name="onescol_r")
        nc.vector.tensor_copy(onescol_r[:], ones_f[:, 0:1])
        ident_h = const.tile([P, P], f16, tag="ident_h", name="ident_h")
        nc.vector.tensor_copy(ident_h[:], ident_f[:])

        def col_load(name, n):
            """1-D DRAM vector [n*128] -> sbuf [128, n] (partition-major)."""
            t = cols.tile([P, n], f32, tag=f"cols_{name}", name=f"cols_{name}")
            for j in range(n):
                nc.sync.dma_start(t[:, j:j + 1], ap[name][ds(P * j, P)])
            return t

        def bcast_rows(tag, row_f, n, psp, pool, rpool=None):
            """[1, n] fp32 row -> [128, n] fp32 tile via ones-matmul."""
            row_r = (rpool or pool).tile([1, n], f32r, tag=f"rr_{tag}",
                                         name=f"rr_{tag}")
            nc.vector.tensor_copy(row_r[:], row_f[0:1, 0:n])
            t = pool.tile([P, n], f32, tag=f"bc_{tag}", name=f"bc_{tag}")
            for h in range(0, n, GS):
                w = min(GS, n - h)
                pt = psp.tile([P, GS], f32, tag="bc_ps", name="bc_ps")
                nc.tensor.matmul(pt[:, 0:w], ones1_r[:], row_r[0:1, h:h + w],
                                 start=True, stop=True)
                nc.scalar.copy(t[:, h:h + w], pt[:, 0:w])
            return t

        bq_c = col_load("bq", DC)
        bk_c = col_load("bk", DC)
        Fb_c = col_load("Fb", KC)
        m1b_c = col_load("m1b", MC)

        bc4 = ctx.enter_context(tc.tile_pool(name="bc4", bufs=1))
        cs_row = rows.tile([1, 2 * K], f32, tag="cs", name="cs")

        # =========== phase A: conditioning vectors ===========
        with (
            tc.tile_pool(name="vec_sb", bufs=3) as vsb,
            tc.tile_pool(name="arow", bufs=1) as arow,
            tc.tile_pool(name="vec_ps", bufs=2, space="PSUM") as vps,
        ):
            def a_row_load(name, n):
                t = arow.tile([1, n], f32, tag=f"row_{name}", name=f"row_{name}")
                nc.sync.dma_start(t[:], ap[name][0:n])
                return t

            h1b_row = a_row_load("h1b", D)
            g1b_row = a_row_load("g1b", D)
            be1b_row = a_row_load("be1b", D)
            h2b_row = a_row_load("h2b", D)
            g2b_row = a_row_load("g2b", D)
            be2b_row = a_row_load("be2b", D)

            zc_f = cols.tile([P, DC], f32, tag="zc_f", name="zc_f")
            for j in range(DC):
                nc.sync.dma_start(zc_f[:, j:j + 1], ap["z"][0:1, ds(P * j, P)])
            zc_r = cols.tile([P, DC], f32r, tag="zc_r", name="zc_r")
            nc.vector.tensor_copy(zc_r[:], zc_f[:])

            def vec_layer(wname, lhs_cols, bias_row, act, out_row):
                """out_row[1, D] = act(lhs^T @ w + bias)."""
                pts = [vps.tile([1, GS], f32, tag=f"vps{h}", name=f"vps{h}")
                       for h in range(2)]
                for j in range(DC):
                    wt = vsb.tile([P, D], f32, tag="vw_f", name="vw_f")
                    nc.sync.dma_start(wt[:], ap[wname][ds(P * j, P), :])
                    wr = vsb.tile([P, D], f32r, tag="vw_r", name="vw_r")
                    nc.scalar.copy(wr[:], wt[:])
                    for h in range(2):
                        nc.tensor.matmul(pts[h][:], lhs_cols[:, j:j + 1],
                                         wr[:, ds(GS * h, GS)],
                                         start=(j == 0), stop=(j == DC - 1))
                for h in range(2):
                    pre = arow.tile([1, GS], f32, tag=f"vpre{h}", name=f"vpre{h}")
                    nc.vector.tensor_add(pre[:], pts[h][:],
                                         bias_row[0:1, ds(GS * h, GS)])
                    if act is None:
                        nc.vector.tensor_copy(out_row[0:1, ds(GS * h, GS)], pre[:])
                    else:
                        nc.scalar.activation(out_row[0:1, ds(GS * h, GS)],
                                             pre[:], act)

            def row_to_cols_r(tag, row_f):
                cf = cols.tile([P, DC], f32, tag=f"c_{tag}", name=f"c_{tag}")
                for j in range(DC):
                    nc.sync.dma_start(cf[:, j:j + 1], row_f[0:1, ds(P * j, P)])
                cr = cols.tile([P, DC], f32r, tag=f"cr_{tag}", name=f"cr_{tag}")
                nc.vector.tensor_copy(cr[:], cf[:])
                return cr

            h1_row = arow.tile([1, D], f32, tag="h1", name="h1")
            h2_row = arow.tile([1, D], f32, tag="h2", name="h2")
            sc1_row = arow.tile([1, D], f32, tag="sc1", name="sc1")
            of1_row = arow.tile([1, D], f32, tag="of1", name="of1")
            sc2_row = arow.tile([1, D], f32, tag="sc2", name="sc2")
            of2_row = arow.tile([1, D], f32, tag="of2", name="of2")
            vec_layer("h1w", zc_r, h1b_row, AF.Silu, h1_row)
            h1_c = row_to_cols_r("h1", h1_row)
            vec_layer("g1w", h1_c, g1b_row, None, sc1_row)
            vec_layer("be1w", h1_c, be1b_row, None, of1_row)
            vec_layer("h2w", zc_r, h2b_row, AF.Silu, h2_row)
            h2_c = row_to_cols_r("h2", h2_row)
            vec_layer("g2w", h2_c, g2b_row, None, sc2_row)
            vec_layer("be2w", h2_c, be2b_row, None, of2_row)
            scale1_b = bcast_rows("s1", sc1_row, D, vps, bc4, arow)
            offset1_b = bcast_rows("o1", of1_row, D, vps, bc4, arow)
            scale2_b = bcast_rows("s2", sc2_row, D, vps, bc4, arow)
            offset2_b = bcast_rows("o2", of2_row, D, vps, bc4, arow)

        # manual pool stacks (LIFO per SBUF side)
        s_qT = contextlib.ExitStack()    # left
        s_kv = contextlib.ExitStack()    # left
        s_pef = contextlib.ExitStack()   # right
        s_aoT = contextlib.ExitStack()   # right
        try:
            # ===== phase B: adaLN1 + transposes + qT =====
            qT_p = s_qT.enter_context(tc.tile_pool(name="qT", bufs=1))
            qT = [[qT_p.tile([P, GS], f32r, tag=f"qT_{j}_{g}", name=f"qT_{j}_{g}")
                   for g in range(NG)] for j in range(DC)]
            x1n = []
            if True:
                with (
                    tc.tile_pool(name="ln1_sb", bufs=2) as ln_sb,
                    tc.tile_pool(name="x1Trot", bufs=2) as x1t_p,
                    tc.tile_pool(name="wq_st", bufs=12) as wqst,
                    tc.tile_pool(name="tp1_ps", bufs=2, space="PSUM") as ln_ps,
                    tc.tile_pool(name="q_ps", bufs=3, space="PSUM") as qps,
                ):
                    for g in range(NG):
                        x1T_g = [x1t_p.tile([P, GS], f32r, tag=f"x1T_{j}",
                                            name=f"x1T_{j}") for j in range(DC)]
                        for ii in range(4):
                            i = 4 * g + ii
                            xt = ln_sb.tile([P, D], f32, tag="ln_in", name="ln_in")
                            nc.sync.dma_start(xt[:], ap["x"][ds(P * i, P), :])
                            st = ln_sb.tile([P, 2, 6], f32, tag="ln_st",
                                            name="ln_st")
                            nc.vector.bn_stats(st[:, 0, :], xt[:, 0:GS])
                            nc.vector.bn_stats(st[:, 1, :], xt[:, GS:D])
                            mv = ln_sb.tile([P, 2], f32, tag="ln_mv", name="ln_mv")
                            nc.vector.bn_aggr(mv[:], st[:])
                            sd = ln_sb.tile([P, 1], f32, tag="ln_sd", name="ln_sd")
                            nc.scalar.activation(sd[:], mv[:, 1:2], AF.Sqrt,
                                                 bias=eps_t[:])
                            rstd = ln_sb.tile([P, 1], f32, tag="ln_rstd",
                                              name="ln_rstd")
                            nc.vector.reciprocal(rstd[:], sd[:])
                            nc.vector.tensor_scalar(xt[:], xt[:], mv[:, 0:1],
                                                    rstd[:], OP.subtract, OP.mult)
                            nc.vector.tensor_mul(xt[:], xt[:], scale1_b[:])
                            x1t = ln_sb.tile([P, D], f32r, tag="nat",
                                             name="nat")
                            nc.vector.tensor_add(x1t[:], xt[:], offset1_b[:])
                            nc.sync.dma_start(x1_sc[ds(P * i, P), :], x1t[:])
                            for j in range(DC):
                                pt = ln_ps.tile([P, P], f32r, tag="tp_ps",
                                                name="tp_ps")
                                nc.tensor.transpose(pt[:], x1t[:, ds(P * j, P)],
                                                    ident_r[:])
                                nc.vector.tensor_copy(
                                    x1T_g[j][:, ds(P * ii, P)], pt[:])
                        for jo in range(DC):
                            pt = qps.tile([P, GS], f32, tag="q_ps", name="q_ps")
                            for j in range(DC):
                                wt = wqst.tile([P, P], f32, tag="wq_f",
                                               name="wq_f")
                                nc.sync.dma_start(
                                    wt[:], ap["wq"][ds(P * j, P), ds(P * jo, P)])
                                wr = wqst.tile([P, P], f32r, tag="wq_r",
                                               name="wq_r")
                                nc.scalar.copy(wr[:], wt[:])
                                nc.tensor.matmul(pt[:], wr[:], x1T_g[j][:],
                                                 start=(j == 0),
                                                 stop=(j == DC - 1))
                            nc.scalar.activation(qT[jo][g][:], pt[:], AF.Identity,
                                                 bias=bq_c[:, jo:jo + 1])

            # ===== phase B2: P_EF = x1^T @ [Ew|Fw] =====
            pef_sb = s_pef.enter_context(
                tc.tile_pool(name="pef_sb", bufs=1, side="right"))
            pef = [pef_sb.tile([P, 2 * K], f32r, tag=f"pefs{j}", name=f"pefs{j}")
                   for j in range(DC)]
            with (
                tc.tile_pool(name="ef_sb", bufs=3) as efsb,
                tc.tile_pool(name="pef_ps", bufs=1, space="PSUM") as pps,
            ):
                pef_ps = [pps.tile([P, 2 * K], f32, tag=f"pef{j}", name=f"pef{j}")
                          for j in range(DC)]
                for i in range(SC):
                    xn_t = efsb.tile([P, D], f32r, tag="x1s", name="x1s")
                    nc.sync.dma_start(xn_t[:], x1_sc[ds(P * i, P), :])
                    ef_f = efsb.tile([P, 2 * K], f32, tag="ef_f", name="ef_f")
                    nc.sync.dma_start(ef_f[:, 0:K], ap["Ew"][ds(P * i, P), :])
                    nc.sync.dma_start(ef_f[:, K:2 * K], ap["Fw"][ds(P * i, P), :])
                    ef_r = efsb.tile([P, 2 * K], f32r, tag="ef_r", name="ef_r")
                    nc.scalar.copy(ef_r[:], ef_f[:])
                    for j in range(DC):
                        nc.tensor.matmul(pef_ps[j][:], xn_t[:, ds(P * j, P)],
                                         ef_r[:],
                                         start=(i == 0), stop=(i == SC - 1))
                for j in range(DC):
                    nc.scalar.copy(pef[j][:], pef_ps[j][:])

            # ===== phase KV: colsums, k_projT, v_proj_ext =====
            kv_sb = s_kv.enter_context(tc.tile_pool(name="kv_sb", bufs=1))
            kpT = [kv_sb.tile([P, K], f32r, tag=f"kpT{j}", name=f"kpT{j}")
                   for j in range(DC)]
            vpe = [kv_sb.tile([P, 65 * H], f32r, tag=f"vpe{c}", name=f"vpe{c}")
                   for c in range(KC)]
            with (
                tc.tile_pool(name="kv_st", bufs=3) as kvst,
                tc.tile_pool(name="kv_wv", bufs=1) as kvwv,
                tc.tile_pool(name="kv_tmp", bufs=2) as kvt,
                tc.tile_pool(name="kv_bias", bufs=1) as kvb,
                tc.tile_pool(name="cs_ps", bufs=1, space="PSUM") as csps,
                tc.tile_pool(name="kv_ps", bufs=2, space="PSUM") as kvps,
            ):
                cs_ps = csps.tile([1, 2 * K], f32, tag="cs_ps", name="cs_ps")
                for i in range(SC):
                    ef_f = kvst.tile([P, 2 * K], f32, tag="ef_f2", name="ef_f2")
                    nc.sync.dma_start(ef_f[:, 0:K], ap["Ew"][ds(P * i, P), :])
                    nc.sync.dma_start(ef_f[:, K:2 * K], ap["Fw"][ds(P * i, P), :])
                    ef_r = kvst.tile([P, 2 * K], f32r, tag="ef_r2", name="ef_r2")
                    nc.scalar.copy(ef_r[:], ef_f[:])
                    nc.tensor.matmul(cs_ps[:], onescol_r[:], ef_r[:],
                                     start=(i == 0), stop=(i == SC - 1))
                nc.vector.tensor_copy(cs_row[:], cs_ps[:])

                bv_row = kvb.tile([1, D], f32, tag="bv_row", name="bv_row")
                nc.sync.dma_start(bv_row[:], ap["bv"][0:D])
                Eb_row = kvb.tile([1, K], f32, tag="Eb_row", name="Eb_row")
                nc.sync.dma_start(Eb_row[:], ap["Eb"][0:K])
                bv_b = bcast_rows("bv", bv_row, D, kvps, kvb, kvt)
                Eb_b = bcast_rows("Eb", Eb_row, K, kvps, kvb, kvt)
                csE_b = bcast_rows("csE", cs_row, K, kvps, kvb, kvt)
                csF_c = kvb.tile([P, KC], f32, tag="csF_c", name="csF_c")
                for c in range(KC):
                    nc.sync.dma_start(csF_c[:, c:c + 1],
                                      cs_row[0:1, ds(K + P * c, P)])
                kp_bias = []
                for j in range(DC):
                    bt = kvb.tile([P, K], f32, tag=f"kpb{j}", name=f"kpb{j}")
                    nc.vector.tensor_scalar(bt[:], csE_b[:], bk_c[:, j:j + 1],
                                            None, OP.mult)
                    nc.vector.tensor_add(bt[:], bt[:], Eb_b[:])
                    kp_bias.append(bt)
                vp_bias = []
                for c in range(KC):
                    bt = kvb.tile([P, D], f32, tag=f"vpb{c}", name=f"vpb{c}")
                    nc.vector.tensor_scalar(bt[:], bv_b[:], csF_c[:, c:c + 1],
                                            Fb_c[:, c:c + 1], OP.mult, OP.add)
                    vp_bias.append(bt)

                for jo in range(DC):
                    pt = kvps.tile([P, K], f32, tag="kp_ps", name="kp_ps")
                    for j in range(DC):
                        wt = kvst.tile([P, P], f32, tag="wk_f", name="wk_f")
                        nc.sync.dma_start(
                            wt[:], ap["wk"][ds(P * j, P), ds(P * jo, P)])
                        wr = kvst.tile([P, P], f32r, tag="wk_r", name="wk_r")
                        nc.scalar.copy(wr[:], wt[:])
                        nc.tensor.matmul(pt[:], wr[:], pef[j][:, 0:K],
                                         start=(j == 0), stop=(j == DC - 1))
                    nc.vector.tensor_add(kpT[jo][:], pt[:], kp_bias[jo][:])
                for hf in range(2):
                    wvr = []
                    for j in range(DC):
                        wt = kvst.tile([P, GS], f32, tag="wv_f", name="wv_f")
                        nc.sync.dma_start(
                            wt[:], ap["wv"][ds(P * j, P), ds(GS * hf, GS)])
                        wr = kvwv.tile([P, GS], f32r, tag=f"wv_r{j}",
                                       name=f"wv_r{j}")
                        nc.scalar.copy(wr[:], wt[:])
                        wvr.append(wr)
                    for c in range(KC):
                        pt = kvps.tile([P, GS], f32, tag="vp_ps", name="vp_ps")
                        for j in range(DC):
                            nc.tensor.matmul(pt[:], pef[j][:, ds(K + P * c, P)],
                                             wvr[j][:],
                                             start=(j == 0), stop=(j == DC - 1))
                        tmp = kvt.tile([P, GS], f32, tag="vp_tmp", name="vp_tmp")
                        nc.vector.tensor_add(tmp[:], pt[:],
                                             vp_bias[c][:, ds(GS * hf, GS)])
                        for hh in range(8):
                            h = 8 * hf + hh
                            nc.vector.tensor_copy(vpe[c][:, ds(65 * h, 64)],
                                                  tmp[:, ds(64 * hh, 64)])
                for c in range(KC):
                    for h in range(H):
                        nc.vector.tensor_copy(vpe[c][:, ds(65 * h + 64, 1)],
                                              ones_f[:, 0:1])
            s_pef.close()

            # ===== phase C2: attention =====
            aoT_p = s_aoT.enter_context(
                tc.tile_pool(name="aoT", bufs=1, side="right"))
            aoT = [[aoT_p.tile([P, GS], f32r, tag=f"aoT_{j}_{g}",
                               name=f"aoT_{j}_{g}")
                    for g in range(NG)] for j in range(DC)]
            with (
                tc.tile_pool(name="at_sb", bufs=3) as atsb,
                tc.tile_pool(name="den_sb", bufs=2) as densb,
                tc.tile_pool(name="sc_ps", bufs=4, space="PSUM") as scps,
                tc.tile_pool(name="av_ps", bufs=2, space="PSUM") as avps,
                tc.tile_pool(name="bc2_ps", bufs=2, space="PSUM") as bcps,
            ):
                def nrm_flush(p):
                    to, rec_r, j, r0, g = p
                    bpt = bcps.tile([64, GS], f32, tag="bc2", name="bc2")
                    nc.tensor.matmul(bpt[:], ones1_r[0:1, 0:64], rec_r[:],
                                     start=True, stop=True)
                    bsb = atsb.tile([64, GS], f32, tag="bsb", name="bsb")
                    nc.scalar.copy(bsb[:], bpt[:])
                    nc.vector.tensor_mul(aoT[j][g][r0:r0 + 64, :], to[:], bsb[:])

                for g in range(NG):
                    pend = None
                    for h in range(H):
                        j, r0 = h // 2, 64 * (h % 2)
                        exps = []
                        for c in range(KC):
                            spt = scps.tile([P, GS], f32, tag="sc", name="sc")
                            nc.tensor.matmul(spt[:],
                                             kpT[j][r0:r0 + 64, ds(P * c, P)],
                                             qT[j][g][r0:r0 + 64, :],
                                             start=True, stop=True)
                            et = atsb.tile([P, GS], f32r, tag="exp", name="exp")
                            nc.scalar.activation(et[:], spt[:], AF.Exp,
                                                 scale=0.125)
                            exps.append(et)
                        apt = avps.tile([65, GS], f32, tag="av", name="av")
                        for c in range(KC):
                            nc.tensor.matmul(apt[:], vpe[c][:, ds(65 * h, 65)],
                                             exps[c][:],
                                             start=(c == 0), stop=(c == KC - 1))
                        # normalize the PREVIOUS head now -- its reciprocal is
                        # ready, so the broadcast matmul doesn't stall the PE
                        if pend is not None:
                            nrm_flush(pend)
                        to = atsb.tile([64, GS], f32, tag="tmpo", name="tmpo")
                        nc.vector.tensor_copy(to[:], apt[0:64, :])
                        dh = densb.tile([1, GS], f32, tag="denh", name="denh")
                        nc.vector.tensor_copy(dh[:], apt[64:65, :])
                        rec = densb.tile([1, GS], f32, tag="rech", name="rech")
                        nc.vector.reciprocal(rec[:], dh[:])
                        rec_r = densb.tile([1, GS], f32r, tag="rech_r",
                                           name="rech_r")
                        nc.vector.tensor_copy(rec_r[:], rec[:])
                        pend = (to, rec_r, j, r0, g)
                    nrm_flush(pend)
            s_kv.close()
            s_qT.close()

            # ===== phase C3: wo + residual -> attn_sc =====
            with (
                tc.tile_pool(name="wo_sb", bufs=1) as wosb,
                tc.tile_pool(name="c3_sb", bufs=3) as c3sb,
                tc.tile_pool(name="c3_bc", bufs=1) as c3bc,
                tc.tile_pool(name="wo_ps", bufs=3, space="PSUM") as wops,
            ):
                with tc.tile_pool(name="c3_bc_ps", bufs=2, space="PSUM") as cbps:
                    bo_row = c3bc.tile([1, D], f32, tag="bo_row", name="bo_row")
                    nc.sync.dma_start(bo_row[:], ap["bo"][0:D])
                    bo_b = bcast_rows("bo", bo_row, D, cbps, c3bc, c3sb)
                wo_r = []
                for j in range(DC):
                    t = c3sb.tile([P, D], f32, tag="wo_f", name="wo_f")
                    nc.sync.dma_start(t[:], ap["wo"][ds(P * j, P), :])
                    r = wosb.tile([P, D], f32r, tag=f"wo_r{j}", name=f"wo_r{j}")
                    nc.scalar.copy(r[:], t[:])
                    wo_r.append(r)
                for i in range(SC):
                    g, c = i // 4, (i % 4) * P
                    xt = c3sb.tile([P, D], f32, tag="res_x", name="res_x")
                    nc.sync.dma_start(xt[:], ap["x"][ds(P * i, P), :])
                    at = c3sb.tile([P, D], f32, tag="attn_nat", name="attn_nat")
                    for hf in range(2):
                        pt = wops.tile([P, GS], f32, tag="wo_ps", name="wo_ps")
                        for j in range(DC):
                            nc.tensor.matmul(pt[:], aoT[j][g][:, ds(c, P)],
                                             wo_r[j][:, ds(GS * hf, GS)],
                                             start=(j == 0), stop=(j == DC - 1))
                        tm = c3sb.tile([P, GS], f32, tag="wo_tmp", name="wo_tmp")
                        nc.vector.tensor_add(tm[:], pt[:], bo_b[:, ds(GS * hf, GS)])
                        nc.vector.tensor_add(at[:, ds(GS * hf, GS)], tm[:],
                                             xt[:, ds(GS * hf, GS)])
                    nc.sync.dma_start(attn_sc[ds(P * i, P), :], at[:])
            s_aoT.close()

            # ===== phase D+E: adaLN2 -> MLP, fused per token-group =====
            with tc.tile_pool(name="e_bc", bufs=1) as ebc:
                with tc.tile_pool(name="e_bc_ps", bufs=2, space="PSUM") as ebps:
                    m2b_row = ebc.tile([1, D], f32, tag="m2b_row", name="m2b_row")
                    nc.sync.dma_start(m2b_row[:], ap["m2b"][0:D])
                    m2b_b = bcast_rows("m2b", m2b_row, D, ebps, ebc)
                with (
                    tc.tile_pool(name="ln2_sb", bufs=2) as ln2_sb,
                    tc.tile_pool(name="x2Trot", bufs=2) as x2t_p,
                    tc.tile_pool(name="hmid", bufs=1) as hmid_p,
                    tc.tile_pool(name="mst", bufs=24) as mst,
                    tc.tile_pool(name="m2st", bufs=4) as m2st,
                    tc.tile_pool(name="e_sb", bufs=2) as esb,
                    tc.tile_pool(name="tp2_ps", bufs=2, space="PSUM") as ln2_ps,
                    tc.tile_pool(name="m1_ps", bufs=2, space="PSUM") as m1ps,
                    tc.tile_pool(name="m2_ps", bufs=1, space="PSUM") as m2ps,
                ):
                    for g in range(NG):
                        x2T_g = [x2t_p.tile([P, GS], f16, tag=f"x2T_{j}",
                                            name=f"x2T_{j}") for j in range(DC)]
                        for ii in range(4):
                            i = 4 * g + ii
                            xt = ln2_sb.tile([P, D], f32, tag="ln2_in",
                                             name="ln2_in")
                            nc.sync.dma_start(xt[:], attn_sc[ds(P * i, P), :])
                            st = ln2_sb.tile([P, 2, 6], f32, tag="ln2_st",
                                             name="ln2_st")
                            nc.vector.bn_stats(st[:, 0, :], xt[:, 0:GS])
                            nc.vector.bn_stats(st[:, 1, :], xt[:, GS:D])
                            mv = ln2_sb.tile([P, 2], f32, tag="ln2_mv",
                                             name="ln2_mv")
                            nc.vector.bn_aggr(mv[:], st[:])
                            sd = ln2_sb.tile([P, 1], f32, tag="ln2_sd",
                                             name="ln2_sd")
                            nc.scalar.activation(sd[:], mv[:, 1:2], AF.Sqrt,
                                                 bias=eps_t[:])
                            rstd = ln2_sb.tile([P, 1], f32, tag="ln2_rstd",
                                               name="ln2_rstd")
                            nc.vector.reciprocal(rstd[:], sd[:])
                            nc.vector.tensor_scalar(xt[:], xt[:], mv[:, 0:1],
                                                    rstd[:], OP.subtract, OP.mult)
                            nc.vector.tensor_mul(xt[:], xt[:], scale2_b[:])
                            x2t = ln2_sb.tile([P, D], f16, tag="x2nat",
                                              name="x2nat")
                            nc.vector.tensor_add(x2t[:], xt[:], offset2_b[:])
                            for j in range(DC):
                                pt = ln2_ps.tile([P, P], f16, tag="tp2_ps",
                                                 name="tp2_ps")
                                nc.tensor.transpose(pt[:], x2t[:, ds(P * j, P)],
                                                    ident_h[:])
                                nc.vector.tensor_copy(
                                    x2T_g[j][:, ds(P * ii, P)], pt[:])
                        hm = [hmid_p.tile([P, GS], f16, tag=f"hm{m}",
                                          name=f"hm{m}") for m in range(MC)]
                        for half in range(2):
                            m2p = [m2ps.tile([P, GS], f32, tag=f"m2p{ss}",
                                             name=f"m2p{ss}") for ss in range(4)]
                            for m in range(MC):
                                if half == 0:
                                    w1f = [mst.tile([P, P], f32, tag="m1st",
                                                    name="m1st")
                                           for _ in range(DC)]
                                    w1r = [mst.tile([P, P], f16, tag="m1str",
                                                    name="m1str")
                                           for _ in range(DC)]
                                    for j in range(DC):
                                        nc.sync.dma_start(
                                            w1f[j][:],
                                            ap["m1w"][ds(P * j, P), ds(P * m, P)])
                                        nc.scalar.copy(w1r[j][:], w1f[j][:])
                                    pt = m1ps.tile([P, GS], f32, tag="m1p",
                                                   name="m1p")
                                    for j in range(DC):
                                        nc.tensor.matmul(
                                            pt[:], w1r[j][:], x2T_g[j][:],
                                            start=(j == 0), stop=(j == DC - 1))
                                    nc.scalar.activation(hm[m][:], pt[:], AF.Gelu,
                                                         bias=m1b_c[:, m:m + 1])
                                w2f = m2st.tile([P, GS], f32, tag="m2f",
                                                name="m2f")
                                nc.sync.dma_start(
                                    w2f[:], ap["m2w"][ds(P * m, P),
                                                      ds(GS * half, GS)])
                                w2r = m2st.tile([P, GS], f16, tag="m2r",
                                                name="m2r")
                                nc.scalar.copy(w2r[:], w2f[:])
                                for ss in range(4):
                                    nc.tensor.matmul(
                                        m2p[ss][:], hm[m][:, ds(P * ss, P)],
                                        w2r[:],
                                        start=(m == 0), stop=(m == MC - 1))
                            for ss in range(4):
                                i = 4 * g + ss
                                rt = esb.tile([P, GS], f32, tag="res_a",
                                              name="res_a")
                                nc.sync.dma_start(
                                    rt[:], attn_sc[ds(P * i, P),
                                                   ds(GS * half, GS)])
                                tm = esb.tile([P, GS], f32, tag="e_tmp",
                                              name="e_tmp")
                                nc.vector.tensor_add(tm[:], m2p[ss][:],
                                                     m2b_b[:, ds(GS * half, GS)])
                                ot = esb.tile([P, GS], f32, tag="e_out",
                                              name="e_out")
                                nc.vector.tensor_add(ot[:], tm[:], rt[:])
                                nc.sync.dma_start(
                                    out[ds(P * i, P), ds(GS * half, GS)], ot[:])
        finally:
            for s in (s_aoT, s_pef, s_kv, s_qT):
                s.close()


def kernel(**inputs):
    nc = build()
    x = np.ascontiguousarray(inputs["x"], dtype=np.float32)
    z = np.ascontiguousarray(inputs["z"], dtype=np.float32)
    base = {}
    for nm, _, _ in W2D:
        base[nm] = np.ascontiguousarray(inputs[nm], dtype=np.float32)
    for nm, _ in W1D:
        base[nm] = np.ascontiguousarray(inputs[nm], dtype=np.float32)
    in_maps = []
    for c in range(B):
        m = dict(base)
        m["x"] = x[c]
        m["z"] = z[c:c + 1]
        in_maps.append(m)
    res = run_bass_kernel_spmd(nc, in_maps, list(range(B)))
    _cache["last"] = res
    return np.stack([res.results[c]["out"] for c in range(B)], axis=0)


# revision 15
# speedup vs baseline: 1.5219x; 1.3424x over previous
"""DiT block (Linformer attention + adaLN + MLP) on 8 TRN2 NeuronCores.

Sharding: data-parallel over batch (B=8 -> one batch element per core).
All matmuls run in float32r (full PE speed at free-dim>=256, ~1.6e-4 rel err).

Layout per core (S=2048 tokens, D=1024 features):
 - adaLN in natural [S_p, D_f] layout (bn_stats over free dim), PE-transpose
   x1 into x1T [D_p, S_f]; qT = wq^T @ x1T computed per token-group while
   the x1T tiles rotate.
 - Linformer K/V: k_projT = wk^T @ (x1^T @ Ew) -- the S->K projection is
   applied to x1 first (P_EF = x1^T @ [Ew|Fw]), so the wk/wv matmuls shrink
   by S/K = 8x and k,v are never materialized.
 - scores stay transposed: scoresT_h [K_p, S_f]; softmax denominators come
   from a fused ones-column appended to v_proj (65-row attn@V output);
   normalization uses a ones-matmul partition-broadcast of 1/denom.
 - MLP streams m1w/m2w from a pre-rounded fp32r DRAM copy in 4 token-groups.

SBUF pool lifetimes are split across the left/right allocator stacks so the
cross-phase handoff chain stays LIFO per side.
"""
import contextlib

import numpy as np

import concourse.bass as bass
import concourse.mybir as mybir
import concourse.tile as tile
from concourse import bacc
from concourse.bass import ds, ts
from concourse.bass_utils import run_bass_kernel_spmd
from concourse.masks import make_identity

f32 = mybir.dt.float32
f32r = mybir.dt.float32r
f16 = mybir.dt.float16
AF = mybir.ActivationFunctionType
OP = mybir.AluOpType

B, S, D, H, K, MLP, ZD = 8, 2048, 1024, 16, 256, 4096, 1024
DH = D // H      # 64
P = 128
SC = S // P      # 16 token chunks of 128
DC = D // P      # 8 feature chunks of 128
NG = 4           # token groups of 512
GS = 512
MC = MLP // P    # 32
KC = K // P      # 2
EPS = 1e-6

W2D = [("wq", D, D), ("wk", D, D), ("wv", D, D), ("wo", D, D),
       ("Ew", S, K), ("Fw", S, K),
       ("h1w", ZD, D), ("g1w", D, D), ("be1w", D, D),
       ("h2w", ZD, D), ("g2w", D, D), ("be2w", D, D),
       ("m1w", D, MLP), ("m2w", MLP, D)]
W1D = [("bq", D), ("bk", D), ("bv", D), ("bo", D), ("Eb", K), ("Fb", K),
       ("h1b", D), ("g1b", D), ("be1b", D), ("h2b", D), ("g2b", D), ("be2b", D),
       ("m1b", MLP), ("m2b", D)]

_cache = {}


def build():
    if "nc" in _cache:
        return _cache["nc"]
    nc = bacc.Bacc("TRN2", target_bir_lowering=False, debug=False, num_devices=8)
    ap = {}
    ap["x"] = nc.dram_tensor("x", [S, D], f32, kind="ExternalInput").ap()
    ap["z"] = nc.dram_tensor("z", [1, ZD], f32, kind="ExternalInput").ap()
    for nm, a, b in W2D:
        ap[nm] = nc.dram_tensor(nm, [a, b], f32, kind="ExternalInput").ap()
    for nm, a in W1D:
        ap[nm] = nc.dram_tensor(nm, [a], f32, kind="ExternalInput").ap()
    out = nc.dram_tensor("out", [S, D], f32, kind="ExternalOutput").ap()
    with tile.TileContext(nc, trace_sim=False) as tc:
        _emit(nc, tc, ap, out)
    nc.compile()
    _cache["nc"] = nc
    return nc


def _emit(nc, tc, ap, out):
    ctx = contextlib.ExitStack()
    with ctx:
        # ---------- whole-kernel pools ----------
        const = ctx.enter_context(tc.tile_pool(name="const", bufs=1))
        rows = ctx.enter_context(tc.tile_pool(name="rows", bufs=1))
        cols = ctx.enter_context(tc.tile_pool(name="cols", bufs=1))
        dram = ctx.enter_context(tc.tile_pool(name="dram", bufs=1, space="DRAM"))

        attn_sc = dram.tile([S, D], f32, tag="attn_sc", name="attn_sc")
        x1_sc = dram.tile([S, D], f32r, tag="x1_sc", name="x1_sc")

        ident_f = const.tile([P, P], f32, tag="ident_f", name="ident_f")
        make_identity(nc, ident_f)
        ident_r = const.tile([P, P], f32r, tag="ident_r", name="ident_r")
        nc.vector.tensor_copy(ident_r[:], ident_f[:])
        eps_t = const.tile([P, 1], f32, tag="eps", name="eps")
        nc.vector.memset(eps_t[:], EPS)
        ones_f = const.tile([P, 16], f32, tag="ones_f", name="ones_f")
        nc.vector.memset(ones_f[:], 1.0)
        ones1_f = const.tile([1, P], f32, tag="ones1_f", name="ones1_f")
        nc.vector.memset(ones1_f[:], 1.0)
        ones1_r = const.tile([1, P], f32r, tag="ones1_r", name="ones1_r")
        nc.vector.tensor_copy(ones1_r[:], ones1_f[:])
        onescol_r = const.tile([P, 1], f32r, tag="onescol_r", name="onescol_r")
        nc.vector.tensor_copy(onescol_r[:], ones_f[:, 0:1])
        ident_h = const.tile([P, P], f16, tag="ident_h", name="ident_h")
        nc.vector.tensor_copy(ident_h[:], ident_f[:])

        def col_load(name, n):
            """1-D DRAM vector [n*128] -> sbuf [128, n] (partition-major)."""
            t = cols.tile([P, n], f32, tag=f"cols_{name}", name=f"cols_{name}")
            for j in range(n):
                nc.sync.dma_start(t[:, j:j + 1], ap[name][ds(P * j, P)])
            return t

        def bcast_rows(tag, row_f, n, psp, pool, rpool=None):
            """[1, n] fp32 row -> [128, n] fp32 tile via ones-matmul."""
            row_r = (rpool or pool).tile([1, n], f32r, tag=f"rr_{tag}",
                                         name=f"rr_{tag}")
            nc.vector.tensor_copy(row_r[:], row_f[0:1, 0:n])
            t = pool.tile([P, n], f32, tag=f"bc_{tag}", name=f"bc_{tag}")
            for h in range(0, n, GS):
                w = min(GS, n - h)
                pt = psp.tile([P, GS], f32, tag="bc_ps", name="bc_ps")
                nc.tensor.matmul(pt[:, 0:w], ones1_r[:], row_r[0:1, h:h + w],
                                 start=True, stop=True)
                nc.scalar.copy(t[:, h:h + w], pt[:, 0:w])
            return t

        bq_c = col_load("bq", DC)
        bk_c = col_load("bk", DC)
        Fb_c = col_load("Fb", KC)
        m1b_c = col_load("m1b", MC)

        bc4 = ctx.enter_context(tc.tile_pool(name="bc4", bufs=1))
        cs_row = rows.tile([1, 2 * K], f32, tag="cs", name="cs")

        # =========== phase A: conditioning vectors ===========
        with (
            tc.tile_pool(name="vec_sb", bufs=3) as vsb,
            tc.tile_pool(name="arow", bufs=1) as arow,
            tc.tile_pool(name="vec_ps", bufs=2, space="PSUM") as vps,
        ):
            def a_row_load(name, n):
                t = arow.tile([1, n], f32, tag=f"row_{name}", name=f"row_{name}")
                nc.sync.dma_start(t[:], ap[name][0:n])
                return t

            h1b_row = a_row_load("h1b", D)
            g1b_row = a_row_load("g1b", D)
            be1b_row = a_row_load("be1b", D)
            h2b_row = a_row_load("h2b", D)
            g2b_row = a_row_load("g2b", D)
            be2b_row = a_row_load("be2b", D)

            zc_f = cols.tile([P, DC], f32, tag="zc_f", name="zc_f")
            for j in range(DC):
                nc.sync.dma_start(zc_f[:, j:j + 1], ap["z"][0:1, ds(P * j, P)])
            zc_r = cols.tile([P, DC], f32r, tag="zc_r", name="zc_r")
            nc.vector.tensor_copy(zc_r[:], zc_f[:])

            def vec_layer(wname, lhs_cols, bias_row, act, out_row):
                """out_row[1, D] = act(lhs^T @ w + bias)."""
                pts = [vps.tile([1, GS], f32, tag=f"vps{h}", name=f"vps{h}")
                       for h in range(2)]
                for j in range(DC):
                    wt = vsb.tile([P, D], f32, tag="vw_f", name="vw_f")
                    nc.sync.dma_start(wt[:], ap[wname][ds(P * j, P), :])
                    wr = vsb.tile([P, D], f32r, tag="vw_r", name="vw_r")
                    nc.scalar.copy(wr[:], wt[:])
                    for h in range(2):
                        nc.tensor.matmul(pts[h][:], lhs_cols[:, j:j + 1],
                                         wr[:, ds(GS * h, GS)],
                                         start=(j == 0), stop=(j == DC - 1))
                for h in range(2):
                    pre = arow.tile([1, GS], f32, tag=f"vpre{h}", name=f"vpre{h}")
                    nc.vector.tensor_add(pre[:], pts[h][:],
                                         bias_row[0:1, ds(GS * h, GS)])
                    if act is None:
                        nc.vector.tensor_copy(out_row[0:1, ds(GS * h, GS)], pre[:])
                    else:
                        nc.scalar.activation(out_row[0:1, ds(GS * h, GS)],
                                             pre[:], act)

            def row_to_cols_r(tag, row_f):
                cf = cols.tile([P, DC], f32, tag=f"c_{tag}", name=f"c_{tag}")
                for j in range(DC):
                    nc.sync.dma_start(cf[:, j:j + 1], row_f[0:1, ds(P * j, P)])
                cr = cols.tile([P, DC], f32r, tag=f"cr_{tag}", name=f"cr_{tag}")
                nc.vector.tensor_copy(cr[:], cf[:])
                return cr

            h1_row = arow.tile([1, D], f32, tag="h1", name="h1")
            h2_row = arow.tile([1, D], f32, tag="h2", name="h2")
            sc1_row = arow.tile([1, D], f32, tag="sc1", name="sc1")
            of1_row = arow.tile([1, D], f32, tag="of1", name="of1")
            sc2_row = arow.tile([1, D], f32, tag="sc2", name="sc2")
            of2_row = arow.tile([1, D], f32, tag="of2", name="of2")
            vec_layer("h1w", zc_r, h1b_row, AF.Silu, h1_row)
            h1_c = row_to_cols_r("h1", h1_row)
            vec_layer("g1w", h1_c, g1b_row, None, sc1_row)
            vec_layer("be1w", h1_c, be1b_row, None, of1_row)
            vec_layer("h2w", zc_r, h2b_row, AF.Silu, h2_row)
            h2_c = row_to_cols_r("h2", h2_row)
            vec_layer("g2w", h2_c, g2b_row, None, sc2_row)
            vec_layer("be2w", h2_c, be2b_row, None, of2_row)
            scale1_b = bcast_rows("s1", sc1_row, D, vps, bc4, arow)
            offset1_b = bcast_rows("o1", of1_row, D, vps, bc4, arow)
            scale2_b = bcast_rows("s2", sc2_row, D, vps, bc4, arow)
            offset2_b = bcast_rows("o2", of2_row, D, vps, bc4, arow)

        # manual pool stacks (LIFO per SBUF side)
        s_qT = contextlib.ExitStack()    # left
        s_kv = contextlib.ExitStack()    # left
        s_pef = contextlib.ExitStack()   # right
        s_aoT = contextlib.ExitStack()   # right
        try:
            # ===== phase B: adaLN1 + transposes + qT =====
            qT_p = s_qT.enter_context(tc.tile_pool(name="qT", bufs=1))
            qT = [[qT_p.tile([P, GS], f32r, tag=f"qT_{j}_{g}", name=f"qT_{j}_{g}")
                   for g in range(NG)] for j in range(DC)]
            x1n = []
            if True:
                with (
                    tc.tile_pool(name="ln1_sb", bufs=2) as ln_sb,
                    tc.tile_pool(name="x1Trot", bufs=2) as x1t_p,
                    tc.tile_pool(name="wq_sb", bufs=1) as wqsb,
                    tc.tile_pool(name="wq_st", bufs=2) as wqst,
                    tc.tile_pool(name="tp1_ps", bufs=2, space="PSUM") as ln_ps,
                    tc.tile_pool(name="q_ps", bufs=3, space="PSUM") as qps,
                ):
                    wq_r = []
                    for j in range(DC):
                        wt = wqst.tile([P, D], f32, tag="wq_f", name="wq_f")
                        nc.sync.dma_start(wt[:], ap["wq"][ds(P * j, P), :])
                        wr = wqsb.tile([P, D], f32r, tag=f"wq_r{j}",
                                       name=f"wq_r{j}")
                        nc.scalar.copy(wr[:], wt[:])
                        wq_r.append(wr)
                    for g in range(NG):
                        x1T_g = [x1t_p.tile([P, GS], f32r, tag=f"x1T_{j}",
                                            name=f"x1T_{j}") for j in range(DC)]
                        for ii in range(4):
                            i = 4 * g + ii
                            xt = ln_sb.tile([P, D], f32, tag="ln_in", name="ln_in")
                            nc.sync.dma_start(xt[:], ap["x"][ds(P * i, P), :])
                            st = ln_sb.tile([P, 2, 6], f32, tag="ln_st",
                                            name="ln_st")
                            nc.vector.bn_stats(st[:, 0, :], xt[:, 0:GS])
                            nc.vector.bn_stats(st[:, 1, :], xt[:, GS:D])
                            mv = ln_sb.tile([P, 2], f32, tag="ln_mv", name="ln_mv")
                            nc.vector.bn_aggr(mv[:], st[:])
                            sd = ln_sb.tile([P, 1], f32, tag="ln_sd", name="ln_sd")
                            nc.scalar.activation(sd[:], mv[:, 1:2], AF.Sqrt,
                                                 bias=eps_t[:])
                            rstd = ln_sb.tile([P, 1], f32, tag="ln_rstd",
                                              name="ln_rstd")
                            nc.vector.reciprocal(rstd[:], sd[:])
                            nmr = ln_sb.tile([P, 1], f32, tag="ln_nmr",
                                             name="ln_nmr")
                            nc.vector.tensor_scalar(nmr[:], mv[:, 0:1], rstd[:],
                                                    -1.0, OP.mult, OP.mult)
                            xn = ln_sb.tile([P, D], f32, tag="ln_xn", name="ln_xn")
                            nc.scalar.activation(xn[:], xt[:], AF.Identity,
                                                 bias=nmr[:], scale=rstd[:])
                            nc.gpsimd.tensor_mul(xn[:], xn[:], scale1_b[:])
                            x1t = ln_sb.tile([P, D], f32r, tag="nat",
                                             name="nat")
                            nc.vector.tensor_add(x1t[:], xn[:], offset1_b[:])
                            nc.sync.dma_start(x1_sc[ds(P * i, P), :], x1t[:])
                            for j in range(DC):
                                pt = ln_ps.tile([P, P], f32r, tag="tp_ps",
                                                name="tp_ps")
                                nc.tensor.transpose(pt[:], x1t[:, ds(P * j, P)],
                                                    ident_r[:])
                                nc.vector.tensor_copy(
                                    x1T_g[j][:, ds(P * ii, P)], pt[:])
                        for jo in range(DC):
                            pt = qps.tile([P, GS], f32, tag="q_ps", name="q_ps")
                            for j in range(DC):
                                nc.tensor.matmul(pt[:],
                                                 wq_r[j][:, ds(P * jo, P)],
                                                 x1T_g[j][:],
                                                 start=(j == 0),
                                                 stop=(j == DC - 1))
                            nc.scalar.activation(qT[jo][g][:], pt[:], AF.Identity,
                                                 bias=bq_c[:, jo:jo + 1])

            # ===== phase B2: P_EF = x1^T @ [Ew|Fw] =====
            pef_sb = s_pef.enter_context(
                tc.tile_pool(name="pef_sb", bufs=1, side="right"))
            pef = [pef_sb.tile([P, 2 * K], f32r, tag=f"pefs{j}", name=f"pefs{j}")
                   for j in range(DC)]
            with (
                tc.tile_pool(name="ef_sb", bufs=3) as efsb,
                tc.tile_pool(name="pef_ps", bufs=1, space="PSUM") as pps,
            ):
                pef_ps = [pps.tile([P, 2 * K], f32, tag=f"pef{j}", name=f"pef{j}")
                          for j in range(DC)]
                for i in range(SC):
                    xn_t = efsb.tile([P, D], f32r, tag="x1s", name="x1s")
                    nc.sync.dma_start(xn_t[:], x1_sc[ds(P * i, P), :])
                    ef_f = efsb.tile([P, 2 * K], f32, tag="ef_f", name="ef_f")
                    nc.sync.dma_start(ef_f[:, 0:K], ap["Ew"][ds(P * i, P), :])
                    nc.sync.dma_start(ef_f[:, K:2 * K], ap["Fw"][ds(P * i, P), :])
                    ef_r = efsb.tile([P, 2 * K], f32r, tag="ef_r", name="ef_r")
                    nc.scalar.copy(ef_r[:], ef_f[:])
                    for j in range(DC):
                        nc.tensor.matmul(pef_ps[j][:], xn_t[:, ds(P * j, P)],
                                         ef_r[:],
                                         start=(i == 0), stop=(i == SC - 1))
                for j in range(DC):
                    nc.scalar.copy(pef[j][:], pef_ps[j][:])

            # ===== phase KV: colsums, k_projT, v_proj_ext =====
            kv_sb = s_kv.enter_context(tc.tile_pool(name="kv_sb", bufs=1))
            kpT = [kv_sb.tile([P, K], f32r, tag=f"kpT{j}", name=f"kpT{j}")
                   for j in range(DC)]
            vpe = [kv_sb.tile([P, 65 * H], f32r, tag=f"vpe{c}", name=f"vpe{c}")
                   for c in range(KC)]
            with (
                tc.tile_pool(name="kv_st", bufs=3) as kvst,
                tc.tile_pool(name="kv_wv", bufs=1) as kvwv,
                tc.tile_pool(name="kv_tmp", bufs=2) as kvt,
                tc.tile_pool(name="kv_bias", bufs=1) as kvb,
                tc.tile_pool(name="cs_ps", bufs=1, space="PSUM") as csps,
                tc.tile_pool(name="kv_ps", bufs=2, space="PSUM") as kvps,
            ):
                cs_ps = csps.tile([1, 2 * K], f32, tag="cs_ps", name="cs_ps")
                for i in range(SC):
                    ef_f = kvst.tile([P, 2 * K], f32, tag="ef_f2", name="ef_f2")
                    nc.sync.dma_start(ef_f[:, 0:K], ap["Ew"][ds(P * i, P), :])
                    nc.sync.dma_start(ef_f[:, K:2 * K], ap["Fw"][ds(P * i, P), :])
                    ef_r = kvst.tile([P, 2 * K], f32r, tag="ef_r2", name="ef_r2")
                    nc.scalar.copy(ef_r[:], ef_f[:])
                    nc.tensor.matmul(cs_ps[:], onescol_r[:], ef_r[:],
                                     start=(i == 0), stop=(i == SC - 1))
                nc.vector.tensor_copy(cs_row[:], cs_ps[:])

                bv_row = kvb.tile([1, D], f32, tag="bv_row", name="bv_row")
                nc.sync.dma_start(bv_row[:], ap["bv"][0:D])
                Eb_row = kvb.tile([1, K], f32, tag="Eb_row", name="Eb_row")
                nc.sync.dma_start(Eb_row[:], ap["Eb"][0:K])
                bv_b = bcast_rows("bv", bv_row, D, kvps, kvb, kvt)
                Eb_b = bcast_rows("Eb", Eb_row, K, kvps, kvb, kvt)
                csE_b = bcast_rows("csE", cs_row, K, kvps, kvb, kvt)
                csF_c = kvb.tile([P, KC], f32, tag="csF_c", name="csF_c")
                for c in range(KC):
                    nc.sync.dma_start(csF_c[:, c:c + 1],
                                      cs_row[0:1, ds(K + P * c, P)])
                kp_bias = []
                for j in range(DC):
                    bt = kvb.tile([P, K], f32, tag=f"kpb{j}", name=f"kpb{j}")
                    nc.vector.tensor_scalar(bt[:], csE_b[:], bk_c[:, j:j + 1],
                                            None, OP.mult)
                    nc.vector.tensor_add(bt[:], bt[:], Eb_b[:])
                    kp_bias.append(bt)
                vp_bias = []
                for c in range(KC):
                    bt = kvb.tile([P, D], f32, tag=f"vpb{c}", name=f"vpb{c}")
                    nc.vector.tensor_scalar(bt[:], bv_b[:], csF_c[:, c:c + 1],
                                            Fb_c[:, c:c + 1], OP.mult, OP.add)
                    vp_bias.append(bt)

                for jo in range(DC):
                    pt = kvps.tile([P, K], f32, tag="kp_ps", name="kp_ps")
                    for j in range(DC):
                        wt = kvst.tile([P, P], f32, tag="wk_f", name="wk_f")
                        nc.sync.dma_start(
                            wt[:], ap["wk"][ds(P * j, P), ds(P * jo, P)])
                        wr = kvst.tile([P, P], f32r, tag="wk_r", name="wk_r")
                        nc.scalar.copy(wr[:], wt[:])
                        nc.tensor.matmul(pt[:], wr[:], pef[j][:, 0:K],
                                         start=(j == 0), stop=(j == DC - 1))
                    nc.vector.tensor_add(kpT[jo][:], pt[:], kp_bias[jo][:])
                for hf in range(2):
                    wvr = []
                    for j in range(DC):
                        wt = kvst.tile([P, GS], f32, tag="wv_f", name="wv_f")
                        nc.sync.dma_start(
                            wt[:], ap["wv"][ds(P * j, P), ds(GS * hf, GS)])
                        wr = kvwv.tile([P, GS], f32r, tag=f"wv_r{j}",
                                       name=f"wv_r{j}")
                        nc.scalar.copy(wr[:], wt[:])
                        wvr.append(wr)
                    for c in range(KC):
                        pt = kvps.tile([P, GS], f32, tag="vp_ps", name="vp_ps")
                        for j in range(DC):
                            nc.tensor.matmul(pt[:], pef[j][:, ds(K + P * c, P)],
                                             wvr[j][:],
                                             start=(j == 0), stop=(j == DC - 1))
                        tmp = kvt.tile([P, GS], f32, tag="vp_tmp", name="vp_tmp")
                        nc.vector.tensor_add(tmp[:], pt[:],
                                             vp_bias[c][:, ds(GS * hf, GS)])
                        for hh in range(8):
                            h = 8 * hf + hh
                            nc.vector.tensor_copy(vpe[c][:, ds(65 * h, 64)],
                                                  tmp[:, ds(64 * hh, 64)])
                for c in range(KC):
                    for h in range(H):
                        nc.vector.tensor_copy(vpe[c][:, ds(65 * h + 64, 1)],
                                              ones_f[:, 0:1])
            s_pef.close()

            # ===== phase C2: attention =====
            aoT_p = s_aoT.enter_context(
                tc.tile_pool(name="aoT", bufs=1, side="right"))
            aoT = [[aoT_p.tile([P, GS], f32r, tag=f"aoT_{j}_{g}",
                               name=f"aoT_{j}_{g}")
                    for g in range(NG)] for j in range(DC)]
            with (
                tc.tile_pool(name="at_sb", bufs=5) as atsb,
                tc.tile_pool(name="den_sb", bufs=2) as densb,
                tc.tile_pool(name="sc_ps", bufs=4, space="PSUM") as scps,
                tc.tile_pool(name="av_ps", bufs=2, space="PSUM") as avps,
                tc.tile_pool(name="bc2_ps", bufs=2, space="PSUM") as bcps,
            ):
                def nrm_flush(p):
                    to, rec_r, j, r0, g = p
                    bpt = bcps.tile([64, GS], f32, tag="bc2", name="bc2")
                    nc.tensor.matmul(bpt[:], ones1_r[0:1, 0:64], rec_r[:],
                                     start=True, stop=True)
                    bsb = atsb.tile([64, GS], f32, tag="bsb", name="bsb")
                    nc.scalar.copy(bsb[:], bpt[:])
                    nc.vector.tensor_mul(aoT[j][g][r0:r0 + 64, :], to[:], bsb[:])

                def emit_scores(g, h):
                    j, r0 = h // 2, 64 * (h % 2)
                    exps = []
                    for c in range(KC):
                        spt = scps.tile([P, GS], f32, tag="sc", name="sc")
                        nc.tensor.matmul(spt[:],
                                         kpT[j][r0:r0 + 64, ds(P * c, P)],
                                         qT[j][g][r0:r0 + 64, :],
                                         start=True, stop=True)
                        et = atsb.tile([P, GS], f32r, tag="exp", name="exp")
                        nc.scalar.activation(et[:], spt[:], AF.Exp, scale=0.125)
                        exps.append(et)
                    return exps

                for g in range(NG):
                    pend = None
                    exps = emit_scores(g, 0)
                    for h in range(H):
                        j, r0 = h // 2, 64 * (h % 2)
                        nxt = emit_scores(g, h + 1) if h + 1 < H else None
                        apt = avps.tile([65, GS], f32, tag="av", name="av")
                        for c in range(KC):
                            nc.tensor.matmul(apt[:], vpe[c][:, ds(65 * h, 65)],
                                             exps[c][:],
                                             start=(c == 0), stop=(c == KC - 1))
                        # normalize the PREVIOUS head now -- its reciprocal is
                        # ready, so the broadcast matmul doesn't stall the PE
                        if pend is not None:
                            nrm_flush(pend)
                        to = atsb.tile([64, GS], f32, tag="tmpo", name="tmpo")
                        nc.vector.tensor_copy(to[:], apt[0:64, :])
                        dh = densb.tile([1, GS], f32, tag="denh", name="denh")
                        nc.vector.tensor_copy(dh[:], apt[64:65, :])
                        rec = densb.tile([1, GS], f32, tag="rech", name="rech")
                        nc.vector.reciprocal(rec[:], dh[:])
                        rec_r = densb.tile([1, GS], f32r, tag="rech_r",
                                           name="rech_r")
                        nc.vector.tensor_copy(rec_r[:], rec[:])
                        pend = (to, rec_r, j, r0, g)
                        exps = nxt
                    nrm_flush(pend)
            s_kv.close()
            s_qT.close()

            # ===== phase C3: wo + residual -> attn_sc =====
            with (
                tc.tile_pool(name="wo_sb", bufs=1) as wosb,
                tc.tile_pool(name="c3_sb", bufs=3) as c3sb,
                tc.tile_pool(name="c3_bc", bufs=1) as c3bc,
                tc.tile_pool(name="wo_ps", bufs=3, space="PSUM") as wops,
            ):
                with tc.tile_pool(name="c3_bc_ps", bufs=2, space="PSUM") as cbps:
                    bo_row = c3bc.tile([1, D], f32, tag="bo_row", name="bo_row")
                    nc.sync.dma_start(bo_row[:], ap["bo"][0:D])
                    bo_b = bcast_rows("bo", bo_row, D, cbps, c3bc, c3sb)
                wo_r = []
                for j in range(DC):
                    t = c3sb.tile([P, D], f32, tag="wo_f", name="wo_f")
                    nc.sync.dma_start(t[:], ap["wo"][ds(P * j, P), :])
                    r = wosb.tile([P, D], f32r, tag=f"wo_r{j}", name=f"wo_r{j}")
                    nc.scalar.copy(r[:], t[:])
                    wo_r.append(r)
                for i in range(SC):
                    g, c = i // 4, (i % 4) * P
                    xt = c3sb.tile([P, D], f32, tag="res_x", name="res_x")
                    nc.sync.dma_start(xt[:], ap["x"][ds(P * i, P), :])
                    at = c3sb.tile([P, D], f32, tag="attn_nat", name="attn_nat")
                    for hf in range(2):
                        pt = wops.tile([P, GS], f32, tag="wo_ps", name="wo_ps")
                        for j in range(DC):
                            nc.tensor.matmul(pt[:], aoT[j][g][:, ds(c, P)],
                                             wo_r[j][:, ds(GS * hf, GS)],
                                             start=(j == 0), stop=(j == DC - 1))
                        tm = c3sb.tile([P, GS], f32, tag="wo_tmp", name="wo_tmp")
                        nc.vector.tensor_add(tm[:], pt[:], bo_b[:, ds(GS * hf, GS)])
                        nc.vector.tensor_add(at[:, ds(GS * hf, GS)], tm[:],
                                             xt[:, ds(GS * hf, GS)])
                    nc.sync.dma_start(attn_sc[ds(P * i, P), :], at[:])
            s_aoT.close()

            # ===== phase D+E: adaLN2 -> MLP, fused per token-group =====
            with tc.tile_pool(name="e_bc", bufs=1) as ebc:
                with tc.tile_pool(name="e_bc_ps", bufs=2, space="PSUM") as ebps:
                    m2b_row = ebc.tile([1, D], f32, tag="m2b_row", name="m2b_row")
                    nc.sync.dma_start(m2b_row[:], ap["m2b"][0:D])
                    m2b_b = bcast_rows("m2b", m2b_row, D, ebps, ebc)
                with (
                    tc.tile_pool(name="ln2_sb", bufs=2) as ln2_sb,
                    tc.tile_pool(name="x2Trot", bufs=2) as x2t_p,
                    tc.tile_pool(name="hmid", bufs=1) as hmid_p,
                    tc.tile_pool(name="mst", bufs=16) as mst,
                    tc.tile_pool(name="m2st", bufs=6) as m2st,
                    tc.tile_pool(name="e_sb", bufs=2) as esb,
                    tc.tile_pool(name="tp2_ps", bufs=2, space="PSUM") as ln2_ps,
                    tc.tile_pool(name="m1_ps", bufs=2, space="PSUM") as m1ps,
                    tc.tile_pool(name="m2_ps", bufs=1, space="PSUM") as m2ps,
                ):
                    for g in range(NG):
                        x2T_g = [x2t_p.tile([P, GS], f16, tag=f"x2T_{j}",
                                            name=f"x2T_{j}") for j in range(DC)]
                        for ii in range(4):
                            i = 4 * g + ii
                            xt = ln2_sb.tile([P, D], f32, tag="ln2_in",
                                             name="ln2_in")
                            nc.sync.dma_start(xt[:], attn_sc[ds(P * i, P), :])
                            st = ln2_sb.tile([P, 2, 6], f32, tag="ln2_st",
                                             name="ln2_st")
                            nc.vector.bn_stats(st[:, 0, :], xt[:, 0:GS])
                            nc.vector.bn_stats(st[:, 1, :], xt[:, GS:D])
                            mv = ln2_sb.tile([P, 2], f32, tag="ln2_mv",
                                             name="ln2_mv")
                            nc.vector.bn_aggr(mv[:], st[:])
                            sd = ln2_sb.tile([P, 1], f32, tag="ln2_sd",
                                             name="ln2_sd")
                            nc.scalar.activation(sd[:], mv[:, 1:2], AF.Sqrt,
                                                 bias=eps_t[:])
                            rstd = ln2_sb.tile([P, 1], f32, tag="ln2_rstd",
                                               name="ln2_rstd")
                            nc.vector.reciprocal(rstd[:], sd[:])
                            nmr = ln2_sb.tile([P, 1], f32, tag="ln2_nmr",
                                              name="ln2_nmr")
                            nc.vector.tensor_scalar(nmr[:], mv[:, 0:1], rstd[:],
                                                    -1.0, OP.mult, OP.mult)
                            xn = ln2_sb.tile([P, D], f32, tag="ln2_xn",
                                             name="ln2_xn")
                            nc.scalar.activation(xn[:], xt[:], AF.Identity,
                                                 bias=nmr[:], scale=rstd[:])
                            nc.gpsimd.tensor_mul(xn[:], xn[:], scale2_b[:])
                            x2t = ln2_sb.tile([P, D], f16, tag="x2nat",
                                              name="x2nat")
                            nc.vector.tensor_add(x2t[:], xn[:], offset2_b[:])
                            for j in range(DC):
                                pt = ln2_ps.tile([P, P], f16, tag="tp2_ps",
                                                 name="tp2_ps")
                                nc.tensor.transpose(pt[:], x2t[:, ds(P * j, P)],
                                                    ident_h[:])
                                nc.vector.tensor_copy(
                                    x2T_g[j][:, ds(P * ii, P)], pt[:])
                        hm = [hmid_p.tile([P, GS], f16, tag=f"hm{m}",
                                          name=f"hm{m}") for m in range(MC)]
                        for half in range(2):
                            m2p = [m2ps.tile([P, GS], f32, tag=f"m2p{ss}",
                                             name=f"m2p{ss}") for ss in range(4)]
                            w1blk = None
                            for m in range(MC):
                                if half == 0:
                                    if m % 4 == 0:
                                        w1blk = []
                                        for j in range(DC):
                                            bf = mst.tile([P, GS], f32,
                                                          tag="m1bf",
                                                          name="m1bf")
                                            nc.sync.dma_start(
                                                bf[:],
                                                ap["m1w"][ds(P * j, P),
                                                          ds(P * m, GS)])
                                            br = mst.tile([P, GS], f16,
                                                          tag="m1br",
                                                          name="m1br")
                                            nc.scalar.copy(br[:], bf[:])
                                            w1blk.append(br)
                                    mo = (m % 4) * P
                                    pt = m1ps.tile([P, GS], f32, tag="m1p",
                                                   name="m1p")
                                    for j in range(DC):
                                        nc.tensor.matmul(
                                            pt[:], w1blk[j][:, ds(mo, P)],
                                            x2T_g[j][:],
                                            start=(j == 0), stop=(j == DC - 1))
                                    nc.scalar.activation(hm[m][:], pt[:], AF.Gelu,
                                                         bias=m1b_c[:, m:m + 1])
                                w2f = m2st.tile([P, GS], f32, tag="m2f",
                                                name="m2f")
                                nc.sync.dma_start(
                                    w2f[:], ap["m2w"][ds(P * m, P),
                                                      ds(GS * half, GS)])
                                w2r = m2st.tile([P, GS], f16, tag="m2r",
                                                name="m2r")
                                nc.scalar.copy(w2r[:], w2f[:])
                                for ss in range(4):
                                    nc.tensor.matmul(
                                        m2p[ss][:], hm[m][:, ds(P * ss, P)],
                                        w2r[:],
                                        start=(m == 0), stop=(m == MC - 1))
                            for ss in range(4):
                                i = 4 * g + ss
                                rt = esb.tile([P, GS], f32, tag="res_a",
                                              name="res_a")
                                nc.sync.dma_start(
                                    rt[:], attn_sc[ds(P * i, P),
                                                   ds(GS * half, GS)])
                                tm = esb.tile([P, GS], f32, tag="e_tmp",
                                              name="e_tmp")
                                nc.vector.tensor_add(tm[:], m2p[ss][:],
                                                     m2b_b[:, ds(GS * half, GS)])
                                ot = esb.tile([P, GS], f32, tag="e_out",
                                              name="e_out")
                                nc.vector.tensor_add(ot[:], tm[:], rt[:])
                                nc.sync.dma_start(
                                    out[ds(P * i, P), ds(GS * half, GS)], ot[:])
        finally:
            for s in (s_aoT, s_pef, s_kv, s_qT):
                s.close()


def kernel(**inputs):
    nc = build()
    x = np.ascontiguousarray(inputs["x"], dtype=np.float32)
    z = np.ascontiguousarray(inputs["z"], dtype=np.float32)
    base = {}
    for nm, _, _ in W2D:
        base[nm] = np.ascontiguousarray(inputs[nm], dtype=np.float32)
    for nm, _ in W1D:
        base[nm] = np.ascontiguousarray(inputs[nm], dtype=np.float32)
    in_maps = []
    for c in range(B):
        m = dict(base)
        m["x"] = x[c]
        m["z"] = z[c:c + 1]
        in_maps.append(m)
    res = run_bass_kernel_spmd(nc, in_maps, list(range(B)))
    _cache["last"] = res
    return np.stack([res.results[c]["out"] for c in range(B)], axis=0)
